# revision 1
# baseline (speedup 1.0000x reference)
"""Causal multi-head attention (RoPE) on 8 TRN2 NeuronCores.

Problem: x[2,2048,2048] -> qkv proj -> rope -> causal attention (16 heads,
head_dim 128) -> output proj + bias. Sharding: (batch, head-group) across the
8 cores - core c handles batch c//4 and heads 4*(c%4)..4*(c%4)+3. Each core
computes a partial output projection over its heads' channels; the host sums
the 4 partials per batch and adds b_o.

Fully SBUF-resident pipeline, no DRAM scratch roundtrips. The QKV
contraction runs in 2 passes of 8 c-tiles each (9 rotating x^T slots);
pass-0 partials evict via ACT copy; pass-1 q/k partials merge back into
PSUM with an identity matmul (keeps the DVE stream clear for the rope
chains) and evict via ACT, while v partials merge with a DVE add.
RoPE is applied in place (partition half-swap via SBUF->SBUF DMAs, sign
folded into sinT host-side), with chains emitted inside the attention jb
loop so the per-engine instruction streams interleave. q^T,k^T (all 4
heads) and batched v stay in SBUF through attention; ctx stays in SBUF
through the output projection, which runs fused per 512-token block.

All matmuls keep K (contraction) on partitions:
  - q,k produced transposed [d, tok]; v produced natural [tok, (h,d)]
  - scores computed transposed s^T[tk, tq] (lhsT=k^T tile, rhs=q^T block):
    softmax needs no transposes; exp on ACT; row-sums on DVE (lacc),
    partition-reduced and broadcast back via tiny ones-matmuls
  - AV: ctx^T[d, tq] = v.T @ p^T with PSUM accumulation over tk tiles
  - outproj: y[tok, o] accumulates the 4 heads' ctx^T.T @ W_o^T slices
Matmuls run in float32r (1 cycle/row at N>=256 vs 4 for fp32).
"""
import math

import numpy as np

import concourse.bacc as bacc
import concourse.mybir as mybir
import concourse.tile as tile
from concourse.bass_utils import run_bass_kernel_spmd

P = 128           # partitions / head_dim
T = 2048          # context length
C = 2048          # d_model
NKT = C // P      # 16 contraction tiles
NTT = T // P      # 16 token tiles
NB = T // 512     # 4 query blocks of 512
HPC = 4           # heads per core
NPASS = 2         # contraction passes
KPP = NKT // NPASS  # c-tiles per pass
NCORES = 8
SCALE = 1.0 / math.sqrt(P)
MASK_NEG = -1.0e30

F32 = mybir.dt.float32
F32R = mybir.dt.float32r
EXP = mybir.ActivationFunctionType.Exp
MULT = mybir.AluOpType.mult
ADD = mybir.AluOpType.add

_CACHE = {}


def _build(matmul_dt=F32R):
    nc = bacc.Bacc("TRN2", target_bir_lowering=False, debug=False,
                   num_devices=NCORES)
    dt = matmul_dt
    xT = nc.dram_tensor("xT", (C, T), dt, kind="ExternalInput").ap()
    wq = nc.dram_tensor("wq", (HPC, P, NKT, P), dt, kind="ExternalInput").ap()
    wk = nc.dram_tensor("wk", (HPC, P, NKT, P), dt, kind="ExternalInput").ap()
    wv = nc.dram_tensor("wv", (NKT, P, HPC * P), dt, kind="ExternalInput").ap()
    wo = nc.dram_tensor("wo", (HPC, P, C), dt, kind="ExternalInput").ap()
    cosT = nc.dram_tensor("cosT", (P, T), F32, kind="ExternalInput").ap()
    sinT = nc.dram_tensor("sinT", (P, T), F32, kind="ExternalInput").ap()
    tri = nc.dram_tensor("tri", (P, P), F32, kind="ExternalInput").ap()
    m3 = nc.dram_tensor("m3", (P, 2 * P), F32, kind="ExternalInput").ap()
    ones = nc.dram_tensor("ones", (P, P), dt, kind="ExternalInput").ap()
    eye = nc.dram_tensor("eye", (P, P), dt, kind="ExternalInput").ap()
    y = nc.dram_tensor("y", (T, C), F32, kind="ExternalOutput").ap()

    with tile.TileContext(nc) as tc:
        with (
            tc.tile_pool(name="gconst", bufs=1) as gpool,
            tc.tile_pool(name="qkbuf", bufs=1) as qkpool,
            tc.tile_pool(name="vbuf", bufs=1) as vpool,
        ):
            tri_sb = gpool.tile([P, P], F32, tag="tri")
            m3_sb = gpool.tile([P, 2 * P], F32, tag="m3")
            ones_sb = gpool.tile([P, P], dt, tag="ones")
            eye_sb = gpool.tile([P, P], dt, tag="eye")

            # persistent activations (SBUF-resident across phases)
            qk_sb = {}
            for h in range(HPC):
                for part in ("q", "k"):
                    for nb in range(NB):
                        t_ = qkpool.tile([P, 512], dt, tag=f"{part}{h}n{nb}",
                                         name=f"{part}{h}n{nb}_sb")
                        qk_sb[(part, h, nb)] = t_
            v_sb = [vpool.tile([P, HPC * P], dt, tag=f"vb{i}", name=f"v{i}_sb")
                    for i in range(NTT)]

            # ------------- Phase 1: QKV projection (4 passes) + rope --------
            with (
                tc.tile_pool(name="xp", bufs=1) as xpool,
                tc.tile_pool(name="wp", bufs=2) as wpool,
                tc.tile_pool(name="psv", bufs=1, space="PSUM") as psv,
                tc.tile_pool(name="ps1", bufs=2, space="PSUM") as ps1,
            ):
                xt_pref = {}
                for ps in range(NPASS):
                    if ps == 1:
                        nc.sync.dma_start(eye_sb[:], eye)
                        nc.sync.dma_start(tri_sb[:], tri)
                        nc.sync.dma_start(m3_sb[:], m3)
                        nc.sync.dma_start(ones_sb[:], ones)
                    kts = [ps * KPP + j for j in range(KPP)]
                    xt = {}
                    wvt = {}
                    w_tiles = {}

                    def load_w(h, part, wtens):
                        tiles = []
                        for half in range(2):
                            wt = wpool.tile([P, KPP // 2, P], dt, tag="w",
                                            bufs=4, name=f"w_{part}{h}_{half}")
                            nc.sync.dma_start(
                                wt[:], wtens[h][:, ps * KPP + half * (KPP // 2):
                                                ps * KPP + (half + 1) * (KPP // 2), :])
                            tiles.append(wt)
                        w_tiles[(part, h)] = tiles

                    for kt in kts:
                        if kt in xt_pref:
                            xt[kt] = xt_pref.pop(kt)
                        else:
                            x_ = xpool.tile([P, T], dt, tag=f"x{kt % 9}",
                                            bufs=1, name=f"x_{kt}")
                            nc.sync.dma_start(x_[:], xT[kt * P:(kt + 1) * P, :])
                            xt[kt] = x_
                        wv_ = wpool.tile([P, HPC * P], dt, tag=f"wv{kt % KPP}",
                                         bufs=1, name=f"wv_{kt}")
                        nc.sync.dma_start(wv_[:], wv[kt])
                        wvt[kt] = wv_
                        if kt == kts[1]:
                            load_w(0, "q", wq)
                        elif kt == kts[3]:
                            load_w(0, "k", wk)
                        elif ps == 0 and kt == kts[-1]:
                            # prefetch pass-1's first x tile into the spare slot
                            pk = KPP
                            px = xpool.tile([P, T], dt, tag=f"x{pk % 9}",
                                            bufs=1, name=f"x_{pk}")
                            nc.sync.dma_start(px[:], xT[pk * P:(pk + 1) * P, :])
                            xt_pref[pk] = px

                    # v: 4 tok-groups of 4 PSUM banks
                    for vg in range(4):
                        vaccs = []
                        for j, kt in enumerate(kts):
                            for i in range(4):
                                tt = vg * 4 + i
                                if j == 0:
                                    va = psv.tile([P, 512], F32, tag=f"v{i}",
                                                  bufs=1, name=f"va{ps}_{vg}_{i}")
                                    vaccs.append(va)
                                nc.tensor.matmul(
                                    vaccs[i][:],
                                    xt[kt][:, tt * P:(tt + 1) * P], wvt[kt][:],
                                    start=(j == 0), stop=(j == KPP - 1))
                        for i in range(4):
                            tt = vg * 4 + i
                            if ps == 0:
                                nc.scalar.copy(v_sb[tt][:], vaccs[i][:])
                            else:
                                nc.vector.tensor_tensor(
                                    v_sb[tt][:], v_sb[tt][:], vaccs[i][:],
                                    op=ADD)

                    # q,k: 8 (head, part) x 4 token-blocks
                    for h in range(HPC):
                        for part, wtens in (("q", wq), ("k", wk)):
                            if (part, h) not in w_tiles:
                                load_w(h, part, wtens)
                            wts = w_tiles[(part, h)]
                            for nb in range(NB):
                                dst = qk_sb[(part, h, nb)]
                                tsl = slice(nb * 512, (nb + 1) * 512)
                                acc = ps1.tile([P, 512], F32, tag="qk", bufs=4)
                                for j, kt in enumerate(kts):
                                    nc.tensor.matmul(
                                        acc[:], wts[j // (KPP // 2)][:, j % (KPP // 2), :],
                                        xt[kt][:, tsl],
                                        start=(j == 0),
                                        stop=(ps == 0 and j == KPP - 1))
                                if ps == 0:
                                    nc.scalar.copy(dst[:], acc[:])
                                else:
                                    nc.tensor.matmul(acc[:], eye_sb[:], dst[:],
                                                     start=False, stop=True)
                                    nc.scalar.copy(dst[:], acc[:])

            # ------------- Phase 2: attention fused with outproj ------------
            # jb outer / head inner; after each jb the output projection for
            # that token block runs, overlapping the next block's attention.
            with (
                tc.tile_pool(name="wop", bufs=1) as wopool,
                tc.tile_pool(name="ctxp", bufs=2) as ctxpool,
                tc.tile_pool(name="lp", bufs=2) as lpool,
                tc.tile_pool(name="pp", bufs=3) as ppool,
                tc.tile_pool(name="cxs", bufs=2) as cxspool,
                tc.tile_pool(name="yp", bufs=4) as ypool,
                tc.tile_pool(name="ps2s", bufs=4, space="PSUM") as ps2s,
                tc.tile_pool(name="ps2c", bufs=1, space="PSUM") as ps2c,
                tc.tile_pool(name="ps2l", bufs=1, space="PSUM") as ps2l,
                tc.tile_pool(name="ps3", bufs=2, space="PSUM") as ps3,
            ):
                # rope setup: rotate_half sign folded into sinT on host;
                # the half-swap is two SBUF->SBUF DMAs. Chains are emitted
                # inside the jb loop so per-engine streams interleave. Only
                # the nb=0 cos/sin chunks load before jb0's rope; wo and the
                # remaining chunks queue behind jb0's swaps.
                rope_cm1 = tc.tile_pool(name="rconst", bufs=1)
                rpool = rope_cm1.__enter__()
                rope_cm2 = tc.tile_pool(name="st", bufs=2)
                spool = rope_cm2.__enter__()
                cos_sb, sin_sb = [], []

                def load_cs(cnb):
                    csl = slice(cnb * 512, (cnb + 1) * 512)
                    c_ = rpool.tile([P, 512], F32, tag=f"cos{cnb}", name=f"cos{cnb}")
                    nc.sync.dma_start(c_[:], cosT[:, csl])
                    cos_sb.append(c_)
                    s_ = rpool.tile([P, 512], F32, tag=f"sin{cnb}", name=f"sin{cnb}")
                    nc.sync.dma_start(s_[:], sinT[:, csl])
                    sin_sb.append(s_)

                load_cs(0)
                half = P // 2
                wo_sb = []

                def rope_chunk(part, h, nb):
                    src = qk_sb[(part, h, nb)]
                    tmp = spool.tile([P, 512], dt, tag="rt", bufs=2, name="tmp")
                    nc.sync.dma_start(tmp[0:half, :], src[half:P, :])
                    nc.sync.dma_start(tmp[half:P, :], src[0:half, :])
                    t1 = spool.tile([P, 512], F32, tag="t1", bufs=2)
                    nc.gpsimd.tensor_tensor(t1[:], src[:], cos_sb[nb][:], op=MULT)
                    t2 = spool.tile([P, 512], F32, tag="t2", bufs=2)
                    nc.vector.tensor_tensor(t2[:], tmp[:], sin_sb[nb][:], op=MULT)
                    nc.vector.tensor_tensor(src[:], t1[:], t2[:], op=ADD)

                for jb in range(NB):
                    for h in range(HPC):
                        rope_chunk("k", h, jb)
                        rope_chunk("q", h, jb)
                    if jb == 0:
                        for cnb in range(1, NB):
                            load_cs(cnb)
                        for h in range(HPC):
                            w_sb = wopool.tile([P, C], dt, tag=f"wo{h}",
                                               name=f"wo{h}_sb")
                            nc.sync.dma_start(w_sb[:], wo[h])
                            wo_sb.append(w_sb)

                    nt = 4 * (jb + 1)
                    qsl = slice(jb * 512, (jb + 1) * 512)
                    ctx_tiles = {}
                    for h in range(HPC):
                        qT_sb = qk_sb[("q", h, jb)]
                        ctx_ps = ps2c.tile([P, 512], F32, tag="ctx", bufs=1)
                        lps = ps2l.tile([1, 512], F32, tag="l", bufs=1)
                        for i in range(nt):
                            r = i - 4 * jb
                            # causal narrowing: only tq >= tk contribute;
                            # r==3 keeps cols 256: with a memset for 256:384
                            c0 = 0 if r < 1 else (r * P if r <= 2 else 2 * P)
                            osl = slice(c0, 512)
                            kch = qk_sb[("k", h, i // 4)]
                            sps = ps2s.tile([P, 512], F32, tag="s", bufs=4)
                            nc.tensor.matmul(
                                sps[:, osl],
                                kch[:, (i % 4) * P:(i % 4 + 1) * P],
                                qT_sb[:, osl], start=True, stop=True)
                            pt = ppool.tile([P, 512], dt, tag="pt", bufs=6)
                            nc.scalar.activation(pt[:, osl], sps[:, osl], EXP,
                                                 scale=SCALE)
                            if 0 <= r <= 2:
                                dsl = slice(r * P, (r + 1) * P)
                                nc.gpsimd.tensor_tensor(
                                    pt[:, dsl], pt[:, dsl], tri_sb[:], op=MULT)
                            elif r == 3:
                                nc.gpsimd.tensor_tensor(
                                    pt[:, 2 * P:4 * P], pt[:, 2 * P:4 * P],
                                    m3_sb[:], op=MULT)
                            nc.tensor.matmul(
                                ctx_ps[:, osl],
                                v_sb[i][:, h * P:(h + 1) * P], pt[:, osl],
                                start=(i == 0), stop=(i == nt - 1))
                            nc.tensor.matmul(lps[:, osl], ones_sb[:, 0:1],
                                             pt[:, osl],
                                             start=(i == 0), stop=(i == nt - 1))
                        rinv = lpool.tile([1, 512], dt, tag="rinv", bufs=2)
                        with nc.allow_low_precision(reason="softmax 1/l fp32r"):
                            nc.vector.reciprocal(rinv[:], lps[:])
                        bps = ps3.tile([P, 512], F32, tag="y", bufs=2,
                                       name="bps")
                        nc.tensor.matmul(bps[:], ones_sb[0:1, :], rinv[:],
                                         start=True, stop=True)
                        cvt = cxspool.tile([P, 512], F32, tag="cvt")
                        nc.vector.tensor_copy(cvt[:], ctx_ps[:])
                        ctx_sb = ctxpool.tile([P, 512], dt, tag=f"cx{h}",
                                              bufs=2, name=f"ctx{h}_{jb}")
                        nc.vector.tensor_tensor(ctx_sb[:], cvt[:], bps[:], op=MULT)
                        ctx_tiles[h] = ctx_sb

                    # outproj for this token block
                    for sub in range(4):
                        tt = jb * 4 + sub
                        ssl = slice(sub * P, (sub + 1) * P)
                        for ob in range(NB):
                            yps = ps3.tile([P, 512], F32, tag="y", bufs=2)
                            for h in range(HPC):
                                nc.tensor.matmul(
                                    yps[:], ctx_tiles[h][:, ssl],
                                    wo_sb[h][:, ob * 512:(ob + 1) * 512],
                                    start=(h == 0), stop=(h == HPC - 1))
                            y_sb = ypool.tile([P, 512], F32, tag="ysb", bufs=4)
                            nc.vector.tensor_copy(y_sb[:], yps[:])
                            nc.sync.dma_start(
                                y[tt * P:(tt + 1) * P, ob * 512:(ob + 1) * 512],
                                y_sb[:])
                rope_cm2.__exit__(None, None, None)
                rope_cm1.__exit__(None, None, None)

    nc.compile()
    return nc


def _build_kernel(matmul_dt=F32R):
    key = str(matmul_dt)
    if key not in _CACHE:
        _CACHE[key] = _build(matmul_dt)
    return _CACHE[key]


def _host_constants():
    tri01 = (np.arange(P)[:, None] <= np.arange(P)[None, :]).astype(np.float32)
    m3 = np.concatenate([np.zeros((P, P), np.float32), tri01], axis=1)
    ones = np.ones((P, P), dtype=np.float32)
    eye = np.eye(P, dtype=np.float32)
    return tri01, m3, ones, eye


def prepare_in_maps(x, W_qkv, W_o, cos, sin):
    tri, m3, ones, eye = _host_constants()
    cosT = np.ascontiguousarray(cos.T)
    # rotate_half sign folded in: rows (head dims) 0..63 negated
    sgn = np.where(np.arange(P) < P // 2, -1.0, 1.0).astype(np.float32)
    sinT = np.ascontiguousarray(sin.T * sgn[:, None])

    in_maps = []
    for core in range(NCORES):
        b = core // 4
        hg0 = (core % 4) * HPC
        rows = slice(hg0 * P, (hg0 + HPC) * P)
        xTc = np.ascontiguousarray(x[b].T)
        wq_r = W_qkv[0 * C:1 * C][rows]        # [512, 2048]
        wk_r = W_qkv[1 * C:2 * C][rows]
        wv_r = W_qkv[2 * C:3 * C][rows]
        # (h, c_in_tile, kt, d) from W^T [2048(c), 512(h,d)]
        wq_t = np.ascontiguousarray(
            wq_r.T.reshape(NKT, P, HPC, P).transpose(2, 1, 0, 3))
        wk_t = np.ascontiguousarray(
            wk_r.T.reshape(NKT, P, HPC, P).transpose(2, 1, 0, 3))
        wv_t = np.ascontiguousarray(wv_r.T.reshape(NKT, P, HPC * P))
        wo_t = np.ascontiguousarray(W_o[:, rows].T.reshape(HPC, P, C))
        in_maps.append({
            "xT": xTc, "wq": wq_t, "wk": wk_t, "wv": wv_t, "wo": wo_t,
            "cosT": cosT, "sinT": sinT, "tri": tri, "m3": m3, "ones": ones, "eye": eye,
        })
    return in_maps


def gather(results, b_o):
    y = np.zeros((2, T, C), dtype=np.float32)
    for core in range(NCORES):
        y[core // 4] += results[core]["y"]
    y += np.asarray(b_o, dtype=np.float32)[None, None, :]
    return y


def kernel(x, W_qkv, W_o, b_o, cos, sin):
    x = np.asarray(x, dtype=np.float32)
    W_qkv = np.asarray(W_qkv, dtype=np.float32)
    W_o = np.asarray(W_o, dtype=np.float32)
    cos = np.asarray(cos, dtype=np.float32)
    sin = np.asarray(sin, dtype=np.float32)
    nc = _build_kernel()
    in_maps = prepare_in_maps(x, W_qkv, W_o, cos, sin)
    res = run_bass_kernel_spmd(nc, in_maps, core_ids=list(range(NCORES)))
    return gather(res.results, b_o)



# revision 8
# speedup vs baseline: 1.1544x; 1.1544x over previous
"""Causal multi-head attention (RoPE) on 8 TRN2 NeuronCores.

Problem: x[2,2048,2048] -> qkv proj -> rope -> causal attention (16 heads,
head_dim 128) -> output proj + bias. Sharding: (batch, head-group) across the
8 cores - core c handles batch c//4 and heads 4*(c%4)..4*(c%4)+3. Each core
computes a partial output projection over its heads' channels; the host sums
the 4 partials per batch and adds b_o.

Mixed-precision pipeline (tolerance 2e-2; this lands ~3.5e-3):
  - QKV projection and output projection run in fp8 (e4m3) with a hi/lo
    3-term split (W1X1 + W1X2 + W2X1, weights pre-scaled by 64 into the
    e4m3 normal range) using DoubleRow matmuls: each instruction contracts
    2x128 rows at 0.5 cycles per output column - 2.67x the f32r rate for
    the same accuracy class.
  - Attention (scores, exp, AV) runs in bf16 (1 cyc/col, no 256-col floor,
    so causal narrowing works at 128-col granularity).
  - Softmax row-sums l use pt as the matmul *stationary* operand with a
    [128,1] ones column as the moving operand: cost 1 cycle per tile-chunk
    instead of N. 1/l is transposed back to row form via 4 tiny PE
    transposes + 4 K=1 broadcast matmuls.
  - All evictions/elementwise work spread across Pool/DVE/ACT to keep the
    sidecar engines under the PE roofline.

Layout: all matmuls keep contraction on partitions; q,k produced transposed
[d, tok], v natural [tok, (h,d)]; scores transposed s^T[tk, tq] so softmax
needs no transposes; ctx^T[d, tq] accumulates over tk tiles; outproj
contracts the 4 heads' channels as 2 DoubleRow head-pairs. The output
projection for block jb-1 is interleaved into block jb's attention inner
loops to fill the PE bubbles left by the exp dependency chain.
"""
import math

import numpy as np
import ml_dtypes

import concourse.bacc as bacc
import concourse.mybir as mybir
import concourse.tile as tile
from concourse.bass_utils import run_bass_kernel_spmd

P = 128           # partitions / head_dim
T = 2048          # context length
C = 2048          # d_model
NTT = T // P      # 16 token tiles
NB = T // 512     # 4 query blocks of 512
HPC = 4           # heads per core
NPAIR = C // 256  # 8 DoubleRow contraction pair-chunks
NCORES = 8
WS = 64.0         # fp8 weight pre-scale
SCALE = 1.0 / math.sqrt(P)
ESC = SCALE / (WS * WS)   # exp() scale: scores carry WS^2

F32 = mybir.dt.float32
BF = mybir.dt.bfloat16
F8 = mybir.dt.float8e4
EXP = mybir.ActivationFunctionType.Exp
MULT = mybir.AluOpType.mult
ADD = mybir.AluOpType.add
SUB = mybir.AluOpType.subtract
DR = mybir.MatmulPerfMode.DoubleRow

_CACHE = {}


def _phase1(nc, tc, dram, qk_sb, v_sb):
    """QKV projection: fp8 hi/lo 3-term DoubleRow, 2 passes of 4 pair-chunks,
    pass-1 merges into bf16 SBUF via Pool adds."""
    x1, x2, wq1, wq2, wk1, wk2, wv1, wv2 = dram
    with (
        tc.tile_pool(name="xp", bufs=1) as xpool,
        tc.tile_pool(name="wp", bufs=1) as wpool,
        tc.tile_pool(name="wvp", bufs=1) as wvpool,
        tc.tile_pool(name="psqk", bufs=3, space="PSUM") as psqk,
        tc.tile_pool(name="psv", bufs=2, space="PSUM") as psv,
    ):
        xt = {}
        for j in range(NPAIR):
            for lv, ten in ((1, x1), (2, x2)):
                t_ = xpool.tile([P, 2, T], F8, tag=f"x{lv}_{j}",
                                bufs=1, name=f"x{lv}_{j}")
                nc.sync.dma_start(t_[:], ten[j])
                xt[(lv, j)] = t_
        wv_sb = {}
        for lv, ten in ((1, wv1), (2, wv2)):
            t_ = wvpool.tile([P, NPAIR, 2, HPC * P], F8, tag=f"wv{lv}")
            nc.sync.dma_start(t_[:], ten)
            wv_sb[lv] = t_

        wten = {"q": (wq1, wq2), "k": (wk1, wk2)}
        for ps in range(2):
            js = list(range(4 * ps, 4 * ps + 4))
            jsl = slice(4 * ps, 4 * ps + 4)
            groups = [(h, part) for h in range(HPC) for part in ("q", "k")]
            for gi, (h, part) in enumerate(groups):
                w_sb = {}
                for lv in (1, 2):
                    t_ = wpool.tile([P, 4, 2, P], F8, tag=f"w{gi % 3}_{lv}",
                                    bufs=3, name=f"w{part}{h}p{ps}_{lv}")
                    nc.sync.dma_start(t_[:], wten[part][lv - 1][h][:, jsl])
                    w_sb[lv] = t_
                for nb in range(NB):
                    tsl = slice(nb * 512, (nb + 1) * 512)
                    acc = psqk.tile([P, 512], F32, tag="qk", bufs=3)
                    n = 0
                    for jj, j in enumerate(js):
                        for wl, xl in ((1, 1), (1, 2), (2, 1)):
                            nc.tensor.matmul(
                                acc[:], w_sb[wl][:, jj], xt[(xl, j)][:, :, tsl],
                                start=(n == 0), stop=(n == 11), perf_mode=DR)
                            n += 1
                    dst = qk_sb[(part, h, nb)]
                    if ps == 0:
                        nc.scalar.copy(dst[:], acc[:])
                    else:
                        nc.vector.tensor_tensor(dst[:], dst[:], acc[:], op=ADD)
                # two v token-tiles after each q/k group
                for tt in (2 * gi, 2 * gi + 1):
                    ssl = slice(tt * P, (tt + 1) * P)
                    vacc = psv.tile([P, 512], F32, tag="v", bufs=2)
                    n = 0
                    for jj, j in enumerate(js):
                        for xl, wl in ((1, 1), (1, 2), (2, 1)):
                            nc.tensor.matmul(
                                vacc[:], xt[(xl, j)][:, :, ssl], wv_sb[wl][:, j],
                                start=(n == 0), stop=(n == 11), perf_mode=DR)
                            n += 1
                    if ps == 0:
                        nc.scalar.copy(v_sb[tt][:], vacc[:])
                    else:
                        nc.vector.tensor_tensor(v_sb[tt][:], v_sb[tt][:],
                                                vacc[:], op=ADD)


def _attention_head(nc, pools, qk_sb, v_sb, consts, jb, h, interleave):
    """Scores/exp/mask/AV/l for one (jb, h), with score pipelining and
    outproj interleave. Returns (ctx_ps, l_ps) PSUM tiles."""
    pss, psc, psl, ppool = pools
    tri_sb, onescol_sb = consts
    qT = qk_sb[("q", h, jb)]
    nt = 4 * (jb + 1)
    ctx_ps = psc.tile([P, 512], F32, tag="ctx", bufs=2)
    l_ps = psl.tile([P, 4], F32, tag="l", bufs=1)

    def score(i):
        r = i - 4 * jb
        c0 = max(0, r * P)
        osl = slice(c0, 512)
        sps = pss.tile([P, 512], F32, tag="s", bufs=2)
        kch = qk_sb[("k", h, i // 4)]
        nc.tensor.matmul(sps[:, osl], kch[:, (i % 4) * P:(i % 4 + 1) * P],
                         qT[:, osl], start=True, stop=True)
        pt = ppool.tile([P, 512], BF, tag="pt", bufs=4)
        nc.scalar.activation(pt[:, osl], sps[:, osl], EXP, scale=ESC)
        if r >= 0:
            dsl = slice(r * P, (r + 1) * P)
            nc.gpsimd.tensor_tensor(pt[:, dsl], pt[:, dsl], tri_sb[:], op=MULT)
        return pt, c0

    cur = score(0)
    for i in range(nt):
        pt, c0 = cur
        if i + 1 < nt:
            cur = score(i + 1)
        osl = slice(c0, 512)
        nc.tensor.matmul(ctx_ps[:, osl], v_sb[i][:, h * P:(h + 1) * P],
                         pt[:, osl], start=(i == 0), stop=(i == nt - 1))
        for c in range(4):
            if c * P >= c0:
                nc.tensor.matmul(l_ps[:, c:c + 1], pt[:, c * P:(c + 1) * P],
                                 onescol_sb[:],
                                 start=(i == 0 and c == 0),
                                 stop=(i == nt - 1 and c == 3))
        interleave()
    return ctx_ps, l_ps


def _normalize_ctx(nc, pools, consts, ctx_ps, l_ps, c1t, c2t, sl):
    """1/l -> row layout -> broadcast -> split normalized ctx into fp8
    hi/lo pair slots."""
    psrt, psb, lpool, rrpool, cxspool, cxnpool = pools
    eye_sb, onesrow_sb = consts
    rinv_col = lpool.tile([P, 4], BF, tag="rc", bufs=2)
    with nc.allow_low_precision(reason="softmax 1/l bf16"):
        nc.vector.reciprocal(rinv_col[:], l_ps[:])
    rt_ps = psrt.tile([1, 512], BF, tag="rt", bufs=1)
    for c in range(4):
        nc.tensor.matmul(rt_ps[0:1, c * P:(c + 1) * P], rinv_col[:, c:c + 1],
                         eye_sb[:], is_transpose=True)
    rinv_row = rrpool.tile([1, 512], BF, tag="rr", bufs=2)
    nc.vector.tensor_copy(rinv_row[:], rt_ps[:])
    bps = psb.tile([P, 512], F32, tag="y", bufs=2)
    nc.tensor.matmul(bps[:], onesrow_sb[:], rinv_row[:], start=True, stop=True)
    cvt = cxspool.tile([P, 512], F32, tag="cvt", bufs=2)
    nc.vector.tensor_copy(cvt[:], ctx_ps[:])
    ctxn = cxnpool.tile([P, 512], F32, tag="cxn", bufs=2)
    nc.vector.tensor_tensor(ctxn[:], cvt[:], bps[:], op=MULT)
    nc.gpsimd.tensor_copy(c1t[:, sl], ctxn[:])
    nc.vector.tensor_tensor(c2t[:, sl], ctxn[:], c1t[:, sl], op=SUB)


def _phase2(nc, tc, dram, qk_sb, v_sb, gtiles):
    wo1, wo2, cosT, sinT, tri, eye, onescol, onesrow, y = dram
    tri_sb, eye_sb, onescol_sb, onesrow_sb = gtiles
    with (
        tc.tile_pool(name="rc", bufs=1) as rpool,
        tc.tile_pool(name="st", bufs=2) as spool,
        tc.tile_pool(name="wop", bufs=1) as wopool,
        tc.tile_pool(name="ctx1p", bufs=2) as c1pool,
        tc.tile_pool(name="ctx2p", bufs=2) as c2pool,
        tc.tile_pool(name="cxs", bufs=2) as cxspool,
        tc.tile_pool(name="cxn", bufs=2) as cxnpool,
        tc.tile_pool(name="lsb", bufs=2) as lpool,
        tc.tile_pool(name="rrow", bufs=2) as rrpool,
        tc.tile_pool(name="pp", bufs=4) as ppool,
        tc.tile_pool(name="yp", bufs=3) as ypool,
        tc.tile_pool(name="pss", bufs=2, space="PSUM") as pss,
        tc.tile_pool(name="psc", bufs=2, space="PSUM") as psc,
        tc.tile_pool(name="psm", bufs=1, space="PSUM") as psm,
        tc.tile_pool(name="psyb", bufs=2, space="PSUM") as psyb,
    ):
        cos_sb = rpool.tile([P, T], BF, tag="cos")
        nc.sync.dma_start(cos_sb[:], cosT)
        sin_sb = rpool.tile([P, T], BF, tag="sin")
        nc.sync.dma_start(sin_sb[:], sinT)
        nc.sync.dma_start(tri_sb[:], tri)
        nc.sync.dma_start(eye_sb[:], eye)
        nc.sync.dma_start(onescol_sb[:], onescol)
        nc.sync.dma_start(onesrow_sb[:], onesrow)
        wo_sb = {}
        for hp in range(2):
            for lv, ten in ((1, wo1), (2, wo2)):
                t_ = wopool.tile([P, 2, C], F8, tag=f"wo{hp}_{lv}")
                nc.sync.dma_start(t_[:], ten[hp])
                wo_sb[(hp, lv)] = t_

        half = P // 2

        def rope_chunk(part, h, jb):
            src = qk_sb[(part, h, jb)]
            jsl = slice(jb * 512, (jb + 1) * 512)
            tmp = spool.tile([P, 512], BF, tag="rt", bufs=2)
            nc.sync.dma_start(tmp[0:half, :], src[half:P, :])
            nc.sync.dma_start(tmp[half:P, :], src[0:half, :])
            t1 = spool.tile([P, 512], BF, tag="t1", bufs=2)
            nc.gpsimd.tensor_tensor(t1[:], src[:], cos_sb[:, jsl], op=MULT)
            t2 = spool.tile([P, 512], BF, tag="t2", bufs=2)
            nc.vector.tensor_tensor(t2[:], tmp[:], sin_sb[:, jsl], op=MULT)
            nc.vector.tensor_tensor(src[:], t1[:], t2[:], op=ADD)

        ctx1 = {}   # (jb, hp) -> [P, 2, 512] fp8
        ctx2 = {}

        def outproj_thunk(jb, sub, ob):
            def run():
                tt = jb * 4 + sub
                ssl = slice(sub * P, (sub + 1) * P)
                osl = slice(ob * 512, (ob + 1) * 512)
                yps = psyb.tile([P, 512], F32, tag="y", bufs=2)
                n = 0
                for hp in range(2):
                    for ct, wl in ((ctx1, 1), (ctx1, 2), (ctx2, 1)):
                        nc.tensor.matmul(
                            yps[:], ct[(jb, hp)][:, :, ssl],
                            wo_sb[(hp, wl)][:, :, osl],
                            start=(n == 0), stop=(n == 5), perf_mode=DR)
                        n += 1
                y_sb = ypool.tile([P, 512], BF, tag="ysb", bufs=3)
                nc.scalar.mul(y_sb[:], yps[:], 1.0 / WS)
                nc.sync.dma_start(y[tt * P:(tt + 1) * P, osl], y_sb[:])
            return run

        pending = []
        ahead = {"pend": pending, "it": 0, "kint": 0}

        def interleave():
            ahead["it"] += 1
            if ahead["pend"] and ahead["kint"] and \
                    ahead["it"] % ahead["kint"] == 0:
                ahead["pend"].pop(0)()

        att_pools = (pss, psc, psm, ppool)
        att_consts = (tri_sb, onescol_sb)
        nrm_pools = (psm, psyb, lpool, rrpool, cxspool, cxnpool)
        nrm_consts = (eye_sb, onesrow_sb)

        for jb in range(NB):
            for h in range(HPC):
                rope_chunk("k", h, jb)
                rope_chunk("q", h, jb)

            nt = 4 * (jb + 1)
            ahead["it"] = 0
            ahead["kint"] = (HPC * nt) // len(pending) if pending else 0

            for h in range(HPC):
                ctx_ps, l_ps = _attention_head(
                    nc, att_pools, qk_sb, v_sb, att_consts, jb, h, interleave)
                hp, sl = h // 2, h % 2
                if sl == 0:
                    ctx1[(jb, hp)] = c1pool.tile(
                        [P, 2, 512], F8, tag=f"c1_{hp}", bufs=2,
                        name=f"c1_{jb}_{hp}")
                    ctx2[(jb, hp)] = c2pool.tile(
                        [P, 2, 512], F8, tag=f"c2_{hp}", bufs=2,
                        name=f"c2_{jb}_{hp}")
                _normalize_ctx(nc, nrm_pools, nrm_consts, ctx_ps, l_ps,
                               ctx1[(jb, hp)], ctx2[(jb, hp)], sl)

            while pending:
                pending.pop(0)()
            pending.extend(outproj_thunk(jb, sub, ob)
                           for sub in range(4) for ob in range(4))
            ahead["pend"] = pending

        while pending:
            pending.pop(0)()


def _build():
    nc = bacc.Bacc("TRN2", target_bir_lowering=False, debug=False,
                   num_devices=NCORES)
    x1 = nc.dram_tensor("x1", (NPAIR, P, 2, T), F8, kind="ExternalInput").ap()
    x2 = nc.dram_tensor("x2", (NPAIR, P, 2, T), F8, kind="ExternalInput").ap()
    wq1 = nc.dram_tensor("wq1", (HPC, P, NPAIR, 2, P), F8, kind="ExternalInput").ap()
    wq2 = nc.dram_tensor("wq2", (HPC, P, NPAIR, 2, P), F8, kind="ExternalInput").ap()
    wk1 = nc.dram_tensor("wk1", (HPC, P, NPAIR, 2, P), F8, kind="ExternalInput").ap()
    wk2 = nc.dram_tensor("wk2", (HPC, P, NPAIR, 2, P), F8, kind="ExternalInput").ap()
    wv1 = nc.dram_tensor("wv1", (P, NPAIR, 2, HPC * P), F8, kind="ExternalInput").ap()
    wv2 = nc.dram_tensor("wv2", (P, NPAIR, 2, HPC * P), F8, kind="ExternalInput").ap()
    wo1 = nc.dram_tensor("wo1", (2, P, 2, C), F8, kind="ExternalInput").ap()
    wo2 = nc.dram_tensor("wo2", (2, P, 2, C), F8, kind="ExternalInput").ap()
    cosT = nc.dram_tensor("cosT", (P, T), BF, kind="ExternalInput").ap()
    sinT = nc.dram_tensor("sinT", (P, T), BF, kind="ExternalInput").ap()
    tri = nc.dram_tensor("tri", (P, P), BF, kind="ExternalInput").ap()
    eye = nc.dram_tensor("eye", (P, P), BF, kind="ExternalInput").ap()
    onescol = nc.dram_tensor("onescol", (P, 1), BF, kind="ExternalInput").ap()
    onesrow = nc.dram_tensor("onesrow", (1, P), BF, kind="ExternalInput").ap()
    y = nc.dram_tensor("y", (T, C), BF, kind="ExternalOutput").ap()

    with tile.TileContext(nc) as tc:
        with (
            tc.tile_pool(name="gconst", bufs=1) as gpool,
            tc.tile_pool(name="qkbuf", bufs=1) as qkpool,
            tc.tile_pool(name="vbuf", bufs=1) as vpool,
        ):
            tri_sb = gpool.tile([P, P], BF, tag="tri")
            eye_sb = gpool.tile([P, P], BF, tag="eye")
            onescol_sb = gpool.tile([P, 1], BF, tag="ocol")
            onesrow_sb = gpool.tile([1, P], BF, tag="orow")

            qk_sb = {}
            for h in range(HPC):
                for part in ("q", "k"):
                    for nb in range(NB):
                        qk_sb[(part, h, nb)] = qkpool.tile(
                            [P, 512], BF, tag=f"{part}{h}n{nb}",
                            name=f"{part}{h}n{nb}_sb")
            v_sb = [vpool.tile([P, HPC * P], BF, tag=f"vb{i}", name=f"v{i}_sb")
                    for i in range(NTT)]

            _phase1(nc, tc, (x1, x2, wq1, wq2, wk1, wk2, wv1, wv2),
                    qk_sb, v_sb)
            _phase2(nc, tc, (wo1, wo2, cosT, sinT, tri, eye, onescol,
                             onesrow, y), qk_sb, v_sb,
                    (tri_sb, eye_sb, onescol_sb, onesrow_sb))

    nc.compile()
    return nc


def _build_kernel():
    if "k" not in _CACHE:
        _CACHE["k"] = _build()
    return _CACHE["k"]


E4NP = ml_dtypes.float8_e4m3
BFNP = ml_dtypes.bfloat16


def _split8(a):
    a1 = np.asarray(a, np.float32).astype(E4NP)
    a2 = (np.asarray(a, np.float32) - a1.astype(np.float32)).astype(E4NP)
    return a1, a2


def prepare_in_maps(x, W_qkv, W_o, cos, sin):
    tri01 = (np.arange(P)[:, None] <= np.arange(P)[None, :]).astype(BFNP)
    eye = np.eye(P, dtype=np.float32).astype(BFNP)
    onescol = np.full((P, 1), WS, dtype=np.float32).astype(BFNP)
    onesrow = np.ones((1, P), dtype=np.float32).astype(BFNP)
    cosT = np.ascontiguousarray(cos.T).astype(BFNP)
    sgn = np.where(np.arange(P) < P // 2, -1.0, 1.0).astype(np.float32)
    sinT = np.ascontiguousarray(sin.T * sgn[:, None]).astype(BFNP)

    # per-batch x fp8 pair chunks [NPAIR, P, 2, T]
    xq = {}
    for b in range(2):
        xT = np.ascontiguousarray(x[b].T)
        xs = _split8(xT)
        xq[b] = tuple(
            np.ascontiguousarray(
                a.reshape(NPAIR, 2, P, T).transpose(0, 2, 1, 3))
            for a in xs)

    def wqk_layout(a):   # [C, 512] -> (HPC, P, NPAIR, 2, P)
        return np.ascontiguousarray(
            a.reshape(NPAIR, 2, P, HPC, P).transpose(3, 2, 0, 1, 4))

    def wv_layout(a):    # [C, 512] -> (P, NPAIR, 2, HPC*P)
        return np.ascontiguousarray(
            a.reshape(NPAIR, 2, P, HPC * P).transpose(2, 0, 1, 3))

    in_maps = []
    for core in range(NCORES):
        b = core // 4
        hg0 = (core % 4) * HPC
        rows = slice(hg0 * P, (hg0 + HPC) * P)
        wq_r = WS * W_qkv[0 * C:1 * C][rows].T   # [C, 512]
        wk_r = WS * W_qkv[1 * C:2 * C][rows].T
        wv_r = WS * W_qkv[2 * C:3 * C][rows].T
        wq1, wq2 = (wqk_layout(a) for a in _split8(wq_r))
        wk1, wk2 = (wqk_layout(a) for a in _split8(wk_r))
        wv1, wv2 = (wv_layout(a) for a in _split8(wv_r))
        wo_r = WS * W_o[:, rows].T               # [512, C]
        wo1, wo2 = (
            np.ascontiguousarray(a.reshape(2, 2, P, C).transpose(0, 2, 1, 3))
            for a in _split8(wo_r))
        in_maps.append({
            "x1": xq[b][0], "x2": xq[b][1],
            "wq1": wq1, "wq2": wq2, "wk1": wk1, "wk2": wk2,
            "wv1": wv1, "wv2": wv2, "wo1": wo1, "wo2": wo2,
            "cosT": cosT, "sinT": sinT, "tri": tri01, "eye": eye,
            "onescol": onescol, "onesrow": onesrow,
        })
    return in_maps


def gather(results, b_o):
    y = np.zeros((2, T, C), dtype=np.float32)
    for core in range(NCORES):
        y[core // 4] += np.asarray(results[core]["y"], dtype=np.float32)
    y += np.asarray(b_o, dtype=np.float32)[None, None, :]
    return y


def kernel(x, W_qkv, W_o, b_o, cos, sin):
    x = np.asarray(x, dtype=np.float32)
    W_qkv = np.asarray(W_qkv, dtype=np.float32)
    W_o = np.asarray(W_o, dtype=np.float32)
    cos = np.asarray(cos, dtype=np.float32)
    sin = np.asarray(sin, dtype=np.float32)
    nc = _build_kernel()
    in_maps = prepare_in_maps(x, W_qkv, W_o, cos, sin)
    res = run_bass_kernel_spmd(nc, in_maps, core_ids=list(range(NCORES)))
    return gather(res.results, b_o)


# revision 24
# speedup vs baseline: 1.3723x; 1.1888x over previous
"""Causal multi-head attention (RoPE) on 8 TRN2 NeuronCores.

Problem: x[2,2048,2048] -> qkv proj -> rope -> causal attention (16 heads,
head_dim 128) -> output proj + bias. Sharding: (batch, head-group) across the
8 cores - core c handles batch c//4 and heads 4*(c%4)..4*(c%4)+3. Each core
computes a partial output projection over its heads' channels; the host sums
the 4 partials per batch and adds b_o.

Mixed-precision pipeline (tolerance 2e-2; this lands ~3.5e-3):
  - QKV projection and output projection run in fp8 (e4m3) with a hi/lo
    3-term split (W1X1 + W1X2 + W2X1, weights pre-scaled by 64 into the
    e4m3 normal range) using DoubleRow matmuls: each instruction contracts
    2x128 rows at 0.5 cycles per output column - 2.67x the f32r rate for
    the same accuracy class.
  - Attention (scores, exp, AV) runs in bf16 (1 cyc/col, no 256-col floor,
    so causal narrowing works at 128-col granularity).
  - Softmax row-sums l use pt as the matmul *stationary* operand with a
    [128,1] ones column as the moving operand: cost 1 cycle per tile-chunk
    instead of N. 1/l is transposed back to row form via 4 tiny PE
    transposes + 4 K=1 broadcast matmuls.
  - All evictions/elementwise work spread across Pool/DVE/ACT to keep the
    sidecar engines under the PE roofline.

Layout: all matmuls keep contraction on partitions; q,k produced transposed
[d, tok], v natural [tok, (h,d)]; scores transposed s^T[tk, tq] so softmax
needs no transposes; ctx^T[d, tq] accumulates over tk tiles; outproj
contracts the 4 heads' channels as 2 DoubleRow head-pairs. The output
projection for block jb-1 is interleaved into block jb's attention inner
loops to fill the PE bubbles left by the exp dependency chain.
"""
import math

import numpy as np
import ml_dtypes

import concourse.bacc as bacc
import concourse.mybir as mybir
import concourse.tile as tile
from concourse.bass_utils import run_bass_kernel_spmd

P = 128           # partitions / head_dim
T = 2048          # context length
C = 2048          # d_model
NTT = T // P      # 16 token tiles
NB = T // 512     # 4 query blocks of 512
HPC = 4           # heads per core
NPAIR = C // 256  # 8 DoubleRow contraction pair-chunks
NCORES = 8
WS = 64.0         # fp8 weight pre-scale
SCALE = 1.0 / math.sqrt(P)
ESC = SCALE / (WS * WS)   # exp() scale: scores carry WS^2

F32 = mybir.dt.float32
BF = mybir.dt.bfloat16
F8 = mybir.dt.float8e4
EXP = mybir.ActivationFunctionType.Exp
MULT = mybir.AluOpType.mult
ADD = mybir.AluOpType.add
SUB = mybir.AluOpType.subtract
DR = mybir.MatmulPerfMode.DoubleRow

_CACHE = {}


def _phase1(nc, tc, dram, qk_sb, v_sb, const_loads, rope0, wvpool,
            xtail_loads):
    """QKV projection: fp8 hi/lo 3-term DoubleRow. Pass A covers pair-chunks
    0-1 (so PE starts ~4us in, DMA-paced), pass B covers 2-7; pass-B partials
    merge into bf16 SBUF via DVE adds. rope0(part, h) is invoked after each
    pass-B group so block-0 rope chains run during pass B."""
    x1, x2, wq1, wq2, wk1, wk2, wv1, wv2 = dram
    PASSES = (list(range(0, 2)), list(range(2, 8)))
    with (
        tc.tile_pool(name="xp", bufs=1) as xpool,
        tc.tile_pool(name="wp", bufs=1) as wpool,
        tc.tile_pool(name="psqk", bufs=3, space="PSUM") as psqk,
        tc.tile_pool(name="psv", bufs=2, space="PSUM") as psv,
    ):
        wten = {"q": (wq1, wq2), "k": (wk1, wk2)}
        groups = [(h, part) for h in range(HPC) for part in ("q", "k")]

        def load_w(gi, ps):
            h, part = groups[gi]
            js = PASSES[ps]
            jsl = slice(js[0], js[-1] + 1)
            w_sb = {}
            for lv in (1, 2):
                t_ = wpool.tile([P, len(js), 2, P], F8, tag=f"w{ps}_{gi}_{lv}",
                                bufs=1, name=f"w{part}{h}p{ps}_{lv}")
                nc.sync.dma_start(t_[:], wten[part][lv - 1][h][:, jsl])
                w_sb[lv] = t_
            return w_sb

        wq_pref = {(0, 0): load_w(0, 0)}
        xt = {}

        def load_x(js_):
            for j in js_:
                for lv, ten in ((1, x1), (2, x2)):
                    t_ = xpool.tile([P, 2, T], F8, tag=f"x{lv}_{j}",
                                    bufs=1, name=f"x{lv}_{j}")
                    nc.sync.dma_start(t_[:], ten[j])
                    xt[(lv, j)] = t_

        load_x(PASSES[0])
        for gi in range(1, len(groups)):
            wq_pref[(gi, 0)] = load_w(gi, 0)
        wv_sb = {}
        for lv, ten in ((1, wv1), (2, wv2)):
            t_ = wvpool.tile([P, NPAIR, 2, HPC * P], F8, tag=f"wv{lv}")
            nc.sync.dma_start(t_[:], ten)
            wv_sb[lv] = t_
        load_x(PASSES[1])
        const_loads()
        xtail_loads()
        for gi in range(len(groups)):
            wq_pref[(gi, 1)] = load_w(gi, 1)

        for ps in range(2):
            js = PASSES[ps]
            nmm = 3 * len(js)
            for gi, (h, part) in enumerate(groups):
                w_sb = wq_pref.pop((gi, ps))
                for nb in range(NB):
                    tsl = slice(nb * 512, (nb + 1) * 512)
                    acc = psqk.tile([P, 512], F32, tag="qk", bufs=3)
                    n = 0
                    for wl, xl in ((1, 1), (1, 2), (2, 1)):
                        for jj, j in enumerate(js):
                            nc.tensor.matmul(
                                acc[:], w_sb[wl][:, jj], xt[(xl, j)][:, :, tsl],
                                start=(n == 0), stop=(n == nmm - 1),
                                perf_mode=DR)
                            n += 1
                    dst = qk_sb[(part, h)]
                    if ps == 0:
                        nc.scalar.copy(dst[:, tsl], acc[:])
                    else:
                        nc.vector.tensor_tensor(dst[:, tsl], dst[:, tsl],
                                                acc[:], op=ADD)
                # two v token-tiles after each q/k group (the last four
                # pass-B tiles are deferred into block-0 attention)
                for tt in (2 * gi, 2 * gi + 1):
                    if ps == 1 and tt >= 12:
                        continue
                    ssl = slice(tt * P, (tt + 1) * P)
                    vacc = psv.tile([P, 512], F32, tag="v", bufs=2)
                    n = 0
                    for xl, wl in ((1, 1), (1, 2), (2, 1)):
                        for jj, j in enumerate(js):
                            nc.tensor.matmul(
                                vacc[:], xt[(xl, j)][:, :, ssl], wv_sb[wl][:, j],
                                start=(n == 0), stop=(n == nmm - 1),
                                perf_mode=DR)
                            n += 1
                    if ps == 0:
                        nc.scalar.copy(v_sb[tt][:], vacc[:])
                    else:
                        nc.vector.tensor_tensor(v_sb[tt][:], v_sb[tt][:],
                                                vacc[:], op=ADD)
                if ps == 1:
                    rope0(part, h)
    return wv_sb


def _attention_head(nc, pools, qk_sb, v_sb, consts, jb, h, interleave,
                    head_start=None):
    """Scores/exp/mask/AV/l for one (jb, h), with score pipelining and
    outproj interleave. head_start (the previous head's deferred
    normalize chain) is emitted after this head's first two scores so
    its PE/DVE ops hide behind fresh score work."""
    pss, psc, psl, ppool, rrpool = pools
    tri_sb, onescol_sb = consts
    qT = qk_sb[("q", h)]
    qsl = slice(jb * 512, (jb + 1) * 512)
    nt = 4 * (jb + 1)
    ctx_ps = psc.tile([P, 512], F32, tag="ctx", bufs=2)
    l_ps = psl.tile([1, 512], F32, tag="l", bufs=1)

    def score(i):
        r = i - 4 * jb
        c0 = max(0, r * P)
        osl = slice(c0, 512)
        sps = pss.tile([P, 512], F32, tag="s", bufs=2)
        kT = qk_sb[("k", h)]
        nc.tensor.matmul(sps[:, osl], kT[:, i * P:(i + 1) * P],
                         qT[:, jb * 512 + c0:(jb + 1) * 512],
                         start=True, stop=True)
        pt = ppool.tile([P, 512], BF, tag="pt", bufs=5)
        nc.scalar.activation(pt[:, osl], sps[:, osl], EXP, scale=ESC)
        if r >= 0:
            dsl = slice(r * P, (r + 1) * P)
            nc.gpsimd.tensor_tensor(pt[:, dsl], pt[:, dsl], tri_sb[:], op=MULT)
        return pt, c0

    ahead = 1
    queue = [score(i) for i in range(min(ahead, nt))]
    for i in range(nt):
        pt, c0 = queue.pop(0)
        if i + ahead < nt:
            queue.append(score(i + ahead))
        if i == 0 and head_start is not None:
            head_start()
        osl = slice(c0, 512)
        nc.tensor.matmul(ctx_ps[:, osl], v_sb[i][:, h * P:(h + 1) * P],
                         pt[:, osl], start=(i == 0), stop=(i == nt - 1))
        nc.tensor.matmul(l_ps[:, osl], onescol_sb[:], pt[:, osl],
                         start=(i == 0), stop=(i == nt - 1))
        interleave()
    rinv_row = rrpool.tile([1, 512], BF, tag="rr", bufs=2)
    with nc.allow_low_precision(reason="softmax 1/l bf16"):
        nc.vector.reciprocal(rinv_row[:], l_ps[:])
    return ctx_ps, rinv_row


def _normalize_ctx(nc, pools, consts, ctx_ps, rinv_row, c1t, c2t, sl):
    """broadcast 1/l -> normalize -> split ctx into fp8 hi/lo pair slots."""
    psb, rrpool, cxspool, cxnpool = pools
    onesrow_sb, = consts
    bps = psb.tile([P, 512], F32, tag="b", bufs=1)
    nc.tensor.matmul(bps[:], onesrow_sb[:], rinv_row[:], start=True, stop=True)
    cvt = cxspool.tile([P, 512], F32, tag="cvt", bufs=2)
    nc.vector.tensor_copy(cvt[:], ctx_ps[:])
    ctxn = cxnpool.tile([P, 512], F32, tag="cxn", bufs=2)
    nc.vector.tensor_tensor(ctxn[:], cvt[:], bps[:], op=MULT)
    nc.vector.tensor_copy(c1t[:, sl], ctxn[:])
    nc.vector.tensor_tensor(c2t[:, sl], ctxn[:], c1t[:, sl], op=SUB)


def _phase2(nc, tc, dram, qk_sb, v_sb, gtiles, vdefer):
    wo1, wo2, y = dram
    tri_sb, onescol_sb, onesrow_sb = gtiles
    with (
        tc.tile_pool(name="wop", bufs=1) as wopool,
        tc.tile_pool(name="ctx1p", bufs=2) as c1pool,
        tc.tile_pool(name="ctx2p", bufs=2) as c2pool,
        tc.tile_pool(name="cxs", bufs=2) as cxspool,
        tc.tile_pool(name="cxn", bufs=2) as cxnpool,
        tc.tile_pool(name="rrow", bufs=2) as rrpool,
        tc.tile_pool(name="pp", bufs=4) as ppool,
        tc.tile_pool(name="yp", bufs=3) as ypool,
        tc.tile_pool(name="pss", bufs=2, space="PSUM") as pss,
        tc.tile_pool(name="psm", bufs=1, space="PSUM") as psm,
        tc.tile_pool(name="psc", bufs=2, space="PSUM") as psc,
        tc.tile_pool(name="psb", bufs=1, space="PSUM") as psb,
        tc.tile_pool(name="psy", bufs=2, space="PSUM") as psy,
    ):
        wo_sb = {}

        def load_wo():
            for hp in range(2):
                for lv, ten in ((1, wo1), (2, wo2)):
                    t_ = wopool.tile([P, 2, C], F8, tag=f"wo{hp}_{lv}")
                    nc.sync.dma_start(t_[:], ten[hp])
                    wo_sb[(hp, lv)] = t_

        ctx1 = {}   # (jb, hp) -> [P, 2, 512] fp8
        ctx2 = {}
        yrow = {}

        def outproj_thunk(jb, sub, ob):
            def run():
                tt = jb * 4 + sub
                ssl = slice(sub * P, (sub + 1) * P)
                osl = slice(ob * 512, (ob + 1) * 512)
                yps = psy.tile([P, 512], F32, tag="y", bufs=2)
                n = 0
                for hp in range(2):
                    for ct, wl in ((ctx1, 1), (ctx1, 2), (ctx2, 1)):
                        nc.tensor.matmul(
                            yps[:], ct[(jb, hp)][:, :, ssl],
                            wo_sb[(hp, wl)][:, :, osl],
                            start=(n == 0), stop=(n == 5), perf_mode=DR)
                        n += 1
                if ob == 0:
                    yrow[tt] = ypool.tile([P, T], BF, tag="ysb", bufs=2,
                                          name=f"yrow{tt}")
                y_sb = yrow[tt]
                if (sub + ob) % 2 == 0:
                    nc.vector.tensor_scalar_mul(y_sb[:, osl], yps[:], 1.0 / WS)
                else:
                    nc.scalar.mul(y_sb[:, osl], yps[:], 1.0 / WS)
                if ob == NB - 1:
                    nc.sync.dma_start(y[tt * P:(tt + 1) * P, :], y_sb[:])
            return run

        xtail, wv_sb = vdefer

        def v_thunk(tt):
            def run():
                ssl = slice(tt * P - 3 * 512, tt * P - 3 * 512 + P)
                vacc = psy.tile([P, 512], F32, tag="y", bufs=2, name="vacc")
                n = 0
                for xl, wl in ((1, 1), (1, 2), (2, 1)):
                    for j in range(2, NPAIR):
                        nc.tensor.matmul(
                            vacc[:], xtail[(xl, j)][:, :, ssl], wv_sb[wl][:, j],
                            start=(n == 0), stop=(n == 17), perf_mode=DR)
                        n += 1
                nc.vector.tensor_tensor(v_sb[tt][:], v_sb[tt][:], vacc[:],
                                        op=ADD)
            return run

        pending = [v_thunk(tt) for tt in range(12, NTT)]
        ahead = {"pend": pending, "it": 0, "kint": 0}

        def interleave():
            ahead["it"] += 1
            if ahead["pend"] and ahead["kint"] and \
                    ahead["it"] % ahead["kint"] == 0:
                ahead["pend"].pop(0)()

        att_pools = (pss, psc, psm, ppool, rrpool)
        att_consts = (tri_sb, onescol_sb)
        nrm_pools = (psb, rrpool, cxspool, cxnpool)
        nrm_consts = (onesrow_sb,)

        load_wo()
        xfin = [None]

        for jb in range(NB):
            if xfin[0] is not None:
                xfin[0]()
                xfin[0] = None

            nt = 4 * (jb + 1)
            ahead["it"] = 0
            ahead["kint"] = (HPC * nt) // len(pending) if pending else 0

            fin = None
            for h in range(HPC):
                ctx_ps, rinv_row = _attention_head(
                    nc, att_pools, qk_sb, v_sb, att_consts, jb, h, interleave,
                    head_start=fin)
                hp, sl = h // 2, h % 2
                if sl == 0:
                    ctx1[(jb, hp)] = c1pool.tile(
                        [P, 2, 512], F8, tag=f"c1_{hp}", bufs=2,
                        name=f"c1_{jb}_{hp}")
                    ctx2[(jb, hp)] = c2pool.tile(
                        [P, 2, 512], F8, tag=f"c2_{hp}", bufs=2,
                        name=f"c2_{jb}_{hp}")

                def fin(cp=ctx_ps, rr=rinv_row, c1t=ctx1[(jb, hp)],
                        c2t=ctx2[(jb, hp)], s=sl):
                    _normalize_ctx(nc, nrm_pools, nrm_consts, cp, rr,
                                   c1t, c2t, s)
            if jb + 1 < NB:
                xfin[0] = fin
            else:
                fin()

            while pending:
                pending.pop(0)()
            pending.extend(outproj_thunk(jb, sub, ob)
                           for sub in range(4) for ob in range(4))
            ahead["pend"] = pending

        while pending:
            pending.pop(0)()


def _build():
    nc = bacc.Bacc("TRN2", target_bir_lowering=False, debug=False,
                   num_devices=NCORES)
    x1 = nc.dram_tensor("x1", (NPAIR, P, 2, T), F8, kind="ExternalInput").ap()
    x2 = nc.dram_tensor("x2", (NPAIR, P, 2, T), F8, kind="ExternalInput").ap()
    wq1 = nc.dram_tensor("wq1", (HPC, P, NPAIR, 2, P), F8, kind="ExternalInput").ap()
    wq2 = nc.dram_tensor("wq2", (HPC, P, NPAIR, 2, P), F8, kind="ExternalInput").ap()
    wk1 = nc.dram_tensor("wk1", (HPC, P, NPAIR, 2, P), F8, kind="ExternalInput").ap()
    wk2 = nc.dram_tensor("wk2", (HPC, P, NPAIR, 2, P), F8, kind="ExternalInput").ap()
    wv1 = nc.dram_tensor("wv1", (P, NPAIR, 2, HPC * P), F8, kind="ExternalInput").ap()
    wv2 = nc.dram_tensor("wv2", (P, NPAIR, 2, HPC * P), F8, kind="ExternalInput").ap()
    wo1 = nc.dram_tensor("wo1", (2, P, 2, C), F8, kind="ExternalInput").ap()
    wo2 = nc.dram_tensor("wo2", (2, P, 2, C), F8, kind="ExternalInput").ap()
    cosT = nc.dram_tensor("cosT", (P, T), BF, kind="ExternalInput").ap()
    sinT = nc.dram_tensor("sinT", (P, T), BF, kind="ExternalInput").ap()
    tri = nc.dram_tensor("tri", (P, P), BF, kind="ExternalInput").ap()
    onescol = nc.dram_tensor("onescol", (P, 1), BF, kind="ExternalInput").ap()
    onesrow = nc.dram_tensor("onesrow", (1, P), BF, kind="ExternalInput").ap()
    y = nc.dram_tensor("y", (T, C), BF, kind="ExternalOutput").ap()

    with tile.TileContext(nc) as tc:
        with (
            tc.tile_pool(name="gconst", bufs=1) as gpool,
            tc.tile_pool(name="qkbuf", bufs=1) as qkpool,
            tc.tile_pool(name="vbuf", bufs=1) as vpool,
        ):
            tri_sb = gpool.tile([P, P], BF, tag="tri")
            onescol_sb = gpool.tile([P, 1], BF, tag="ocol")
            onesrow_sb = gpool.tile([1, P], BF, tag="orow")
            cos_sb = gpool.tile([P, T], BF, tag="cos")
            sin_sb = gpool.tile([P, T], BF, tag="sin")

            def const_loads():
                nc.sync.dma_start(cos_sb[:], cosT)
                nc.sync.dma_start(sin_sb[:], sinT)
                nc.sync.dma_start(tri_sb[:], tri)
                nc.sync.dma_start(onescol_sb[:], onescol)
                nc.sync.dma_start(onesrow_sb[:], onesrow)

            wvpool_cm = tc.tile_pool(name="wvp", bufs=1)
            wvpool = wvpool_cm.__enter__()
            xtpool_cm = tc.tile_pool(name="xtail", bufs=1)
            xtpool = xtpool_cm.__enter__()
            xtail = {}

            def xtail_loads():
                for j in range(2, NPAIR):
                    for lv, ten in ((1, x1), (2, x2)):
                        t_ = xtpool.tile([P, 2, 512], F8, tag=f"xt{lv}_{j}",
                                         name=f"xt{lv}_{j}")
                        nc.sync.dma_start(t_[:], ten[j][:, :, 3 * 512:])
                        xtail[(lv, j)] = t_

            spool_cm = tc.tile_pool(name="st", bufs=2)
            spool = spool_cm.__enter__()
            half = P // 2

            def rope_row(part, h):
                # whole-row rope for (part, h): the half-swap is 2 big DMAs
                # instead of 8 small ones (descriptor floor dominates small
                # transfers), and the mul/mul/add chain is 3 ops per row
                src = qk_sb[(part, h)]
                tmp = spool.tile([P, T], BF, tag="rt", bufs=2, name="rtmp")
                t1 = spool.tile([P, T], BF, tag="t1", bufs=2, name="rt1")
                t2 = spool.tile([P, T], BF, tag="t2", bufs=2, name="rt2")
                nc.sync.dma_start(tmp[0:half, :], src[half:P, :])
                nc.sync.dma_start(tmp[half:P, :], src[0:half, :])
                nc.gpsimd.tensor_tensor(t1[:], src[:], cos_sb[:], op=MULT)
                nc.vector.tensor_tensor(t2[:], tmp[:], sin_sb[:], op=MULT)
                nc.vector.tensor_tensor(src[:], t1[:], t2[:], op=ADD)

            qk_sb = {}
            for h in range(HPC):
                for part in ("q", "k"):
                    qk_sb[(part, h)] = qkpool.tile(
                        [P, T], BF, tag=f"{part}{h}",
                        name=f"{part}{h}_sb")
            v_sb = [vpool.tile([P, HPC * P], BF, tag=f"vb{i}", name=f"v{i}_sb")
                    for i in range(NTT)]

            wv_sb = _phase1(nc, tc, (x1, x2, wq1, wq2, wk1, wk2, wv1, wv2),
                            qk_sb, v_sb, const_loads, rope_row, wvpool,
                            xtail_loads)
            spool_cm.__exit__(None, None, None)
            _phase2(nc, tc, (wo1, wo2, y), qk_sb, v_sb,
                    (tri_sb, onescol_sb, onesrow_sb), (xtail, wv_sb))
            xtpool_cm.__exit__(None, None, None)
            wvpool_cm.__exit__(None, None, None)

    nc.compile()
    return nc


def _build_kernel():
    if "k" not in _CACHE:
        _CACHE["k"] = _build()
    return _CACHE["k"]


E4NP = ml_dtypes.float8_e4m3
BFNP = ml_dtypes.bfloat16


def _split8(a):
    a1 = np.asarray(a, np.float32).astype(E4NP)
    a2 = (np.asarray(a, np.float32) - a1.astype(np.float32)).astype(E4NP)
    return a1, a2


def prepare_in_maps(x, W_qkv, W_o, cos, sin):
    tri01 = (np.arange(P)[:, None] <= np.arange(P)[None, :]).astype(BFNP)
    onescol = np.full((P, 1), WS, dtype=np.float32).astype(BFNP)
    onesrow = np.ones((1, P), dtype=np.float32).astype(BFNP)
    cosT = np.ascontiguousarray(cos.T).astype(BFNP)
    sgn = np.where(np.arange(P) < P // 2, -1.0, 1.0).astype(np.float32)
    sinT = np.ascontiguousarray(sin.T * sgn[:, None]).astype(BFNP)

    # per-batch x fp8 pair chunks [NPAIR, P, 2, T]
    xq = {}
    for b in range(2):
        xT = np.ascontiguousarray(x[b].T)
        xs = _split8(xT)
        xq[b] = tuple(
            np.ascontiguousarray(
                a.reshape(NPAIR, 2, P, T).transpose(0, 2, 1, 3))
            for a in xs)

    def wqk_layout(a):   # [C, 512] -> (HPC, P, NPAIR, 2, P)
        return np.ascontiguousarray(
            a.reshape(NPAIR, 2, P, HPC, P).transpose(3, 2, 0, 1, 4))

    def wv_layout(a):    # [C, 512] -> (P, NPAIR, 2, HPC*P)
        return np.ascontiguousarray(
            a.reshape(NPAIR, 2, P, HPC * P).transpose(2, 0, 1, 3))

    in_maps = []
    for core in range(NCORES):
        b = core // 4
        hg0 = (core % 4) * HPC
        rows = slice(hg0 * P, (hg0 + HPC) * P)
        wq_r = WS * W_qkv[0 * C:1 * C][rows].T   # [C, 512]
        wk_r = WS * W_qkv[1 * C:2 * C][rows].T
        wv_r = WS * W_qkv[2 * C:3 * C][rows].T
        wq1, wq2 = (wqk_layout(a) for a in _split8(wq_r))
        wk1, wk2 = (wqk_layout(a) for a in _split8(wk_r))
        wv1, wv2 = (wv_layout(a) for a in _split8(wv_r))
        wo_r = WS * W_o[:, rows].T               # [512, C]
        wo1, wo2 = (
            np.ascontiguousarray(a.reshape(2, 2, P, C).transpose(0, 2, 1, 3))
            for a in _split8(wo_r))
        in_maps.append({
            "x1": xq[b][0], "x2": xq[b][1],
            "wq1": wq1, "wq2": wq2, "wk1": wk1, "wk2": wk2,
            "wv1": wv1, "wv2": wv2, "wo1": wo1, "wo2": wo2,
            "cosT": cosT, "sinT": sinT, "tri": tri01,
            "onescol": onescol, "onesrow": onesrow,
        })
    return in_maps


def gather(results, b_o):
    y = np.zeros((2, T, C), dtype=np.float32)
    for core in range(NCORES):
        y[core // 4] += np.asarray(results[core]["y"], dtype=np.float32)
    y += np.asarray(b_o, dtype=np.float32)[None, None, :]
    return y


def kernel(x, W_qkv, W_o, b_o, cos, sin):
    x = np.asarray(x, dtype=np.float32)
    W_qkv = np.asarray(W_qkv, dtype=np.float32)
    W_o = np.asarray(W_o, dtype=np.float32)
    cos = np.asarray(cos, dtype=np.float32)
    sin = np.asarray(sin, dtype=np.float32)
    nc = _build_kernel()
    in_maps = prepare_in_maps(x, W_qkv, W_o, cos, sin)
    res = run_bass_kernel_spmd(nc, in_maps, core_ids=list(range(NCORES)))
    return gather(res.results, b_o)


# revision 30
# speedup vs baseline: 1.3784x; 1.0044x over previous
"""Causal multi-head attention (RoPE) on 8 TRN2 NeuronCores.

Problem: x[2,2048,2048] -> qkv proj -> rope -> causal attention (16 heads,
head_dim 128) -> output proj + bias. Sharding: (batch, head-group) across the
8 cores - core c handles batch c//4 and heads 4*(c%4)..4*(c%4)+3. Each core
computes a partial output projection over its heads' channels; the host sums
the 4 partials per batch and adds b_o.

Mixed-precision pipeline (tolerance 2e-2; this lands ~3.5e-3):
  - QKV projection and output projection run in fp8 (e4m3) with a hi/lo
    3-term split (W1X1 + W1X2 + W2X1, weights pre-scaled by 64 into the
    e4m3 normal range) using DoubleRow matmuls: each instruction contracts
    2x128 rows at 0.5 cycles per output column - 2.67x the f32r rate for
    the same accuracy class.
  - Attention (scores, exp, AV) runs in bf16 (1 cyc/col, no 256-col floor,
    so causal narrowing works at 128-col granularity).
  - Softmax row-sums l use pt as the matmul *stationary* operand with a
    [128,1] ones column as the moving operand: cost 1 cycle per tile-chunk
    instead of N. 1/l is transposed back to row form via 4 tiny PE
    transposes + 4 K=1 broadcast matmuls.
  - All evictions/elementwise work spread across Pool/DVE/ACT to keep the
    sidecar engines under the PE roofline.

Layout: all matmuls keep contraction on partitions; q,k produced transposed
[d, tok], v natural [tok, (h,d)]; scores transposed s^T[tk, tq] so softmax
needs no transposes; ctx^T[d, tq] accumulates over tk tiles; outproj
contracts the 4 heads' channels as 2 DoubleRow head-pairs. The output
projection for block jb-1 is interleaved into block jb's attention inner
loops to fill the PE bubbles left by the exp dependency chain.
"""
import math

import numpy as np
import ml_dtypes

import concourse.bacc as bacc
import concourse.mybir as mybir
import concourse.tile as tile
from concourse.bass_utils import run_bass_kernel_spmd

P = 128           # partitions / head_dim
T = 2048          # context length
C = 2048          # d_model
NTT = T // P      # 16 token tiles
NB = T // 512     # 4 query blocks of 512
HPC = 4           # heads per core
NPAIR = C // 256  # 8 DoubleRow contraction pair-chunks
NCORES = 8
WS = 64.0         # fp8 weight pre-scale
SCALE = 1.0 / math.sqrt(P)
ESC = SCALE / (WS * WS)   # exp() scale: scores carry WS^2

F32 = mybir.dt.float32
BF = mybir.dt.bfloat16
F8 = mybir.dt.float8e4
EXP = mybir.ActivationFunctionType.Exp
MULT = mybir.AluOpType.mult
ADD = mybir.AluOpType.add
SUB = mybir.AluOpType.subtract
DR = mybir.MatmulPerfMode.DoubleRow

_CACHE = {}


def _phase1(nc, tc, dram, qk_sb, v_sb, const_loads, rope0, wvpool,
            xtail_loads):
    """QKV projection: fp8 hi/lo 3-term DoubleRow. Pass A covers pair-chunks
    0-1 (so PE starts ~4us in, DMA-paced), pass B covers 2-7; pass-B partials
    merge into bf16 SBUF via DVE adds. rope0(part, h) is invoked after each
    pass-B group so block-0 rope chains run during pass B."""
    x1, x2, wq1, wq2, wk1, wk2, wv1, wv2 = dram
    PASSES = (list(range(0, 2)), list(range(2, 8)))
    with (
        tc.tile_pool(name="xp", bufs=1) as xpool,
        tc.tile_pool(name="wp", bufs=1) as wpool,
        tc.tile_pool(name="psqk", bufs=3, space="PSUM") as psqk,
        tc.tile_pool(name="psv", bufs=2, space="PSUM") as psv,
    ):
        wten = {"q": (wq1, wq2), "k": (wk1, wk2)}
        groups = [(h, part) for h in range(HPC) for part in ("q", "k")]

        def load_w(gi, ps):
            h, part = groups[gi]
            js = PASSES[ps]
            jsl = slice(js[0], js[-1] + 1)
            w_sb = {}
            for lv in (1, 2):
                t_ = wpool.tile([P, len(js), 2, P], F8, tag=f"w{ps}_{gi}_{lv}",
                                bufs=1, name=f"w{part}{h}p{ps}_{lv}")
                nc.sync.dma_start(t_[:], wten[part][lv - 1][h][:, jsl])
                w_sb[lv] = t_
            return w_sb

        wq_pref = {(0, 0): load_w(0, 0)}
        xt = {}

        def load_x(js_):
            for j in js_:
                for lv, ten in ((1, x1), (2, x2)):
                    t_ = xpool.tile([P, 2, T], F8, tag=f"x{lv}_{j}",
                                    bufs=1, name=f"x{lv}_{j}")
                    nc.sync.dma_start(t_[:], ten[j])
                    xt[(lv, j)] = t_

        load_x(PASSES[0])
        for gi in range(1, len(groups)):
            wq_pref[(gi, 0)] = load_w(gi, 0)
        wv_sb = {}
        for lv, ten in ((1, wv1), (2, wv2)):
            t_ = wvpool.tile([P, NPAIR, 2, HPC * P], F8, tag=f"wv{lv}")
            nc.sync.dma_start(t_[:], ten)
            wv_sb[lv] = t_
        load_x(PASSES[1])
        const_loads()
        xtail_loads()
        for gi in range(len(groups)):
            wq_pref[(gi, 1)] = load_w(gi, 1)

        for ps in range(2):
            js = PASSES[ps]
            nmm = 3 * len(js)
            for gi, (h, part) in enumerate(groups):
                w_sb = wq_pref.pop((gi, ps))
                for nb in range(NB):
                    tsl = slice(nb * 512, (nb + 1) * 512)
                    acc = psqk.tile([P, 512], F32, tag="qk", bufs=3)
                    n = 0
                    order = ([(jj, t) for jj in range(len(js)) for t in range(3)]
                             if ps == 0 else
                             [(jj, t) for t in range(3) for jj in range(len(js))])
                    terms = ((1, 1), (1, 2), (2, 1))
                    for jj, t in order:
                        wl, xl = terms[t]
                        nc.tensor.matmul(
                            acc[:], w_sb[wl][:, jj], xt[(xl, js[jj])][:, :, tsl],
                            start=(n == 0), stop=(n == nmm - 1),
                            perf_mode=DR)
                        n += 1
                    dst = qk_sb[(part, h)]
                    if ps == 0:
                        nc.scalar.copy(dst[:, tsl], acc[:])
                    else:
                        nc.vector.tensor_tensor(dst[:, tsl], dst[:, tsl],
                                                acc[:], op=ADD)
                # two v token-tiles after each q/k group (the last four
                # pass-B tiles are deferred into block-0 attention)
                for tt in (2 * gi, 2 * gi + 1):
                    if ps == 1 and tt >= 12:
                        continue
                    ssl = slice(tt * P, (tt + 1) * P)
                    vacc = psv.tile([P, 512], F32, tag="v", bufs=2)
                    n = 0
                    for xl, wl in ((1, 1), (1, 2), (2, 1)):
                        for jj, j in enumerate(js):
                            nc.tensor.matmul(
                                vacc[:], xt[(xl, j)][:, :, ssl], wv_sb[wl][:, j],
                                start=(n == 0), stop=(n == nmm - 1),
                                perf_mode=DR)
                            n += 1
                    if ps == 0:
                        nc.scalar.copy(v_sb[tt][:], vacc[:])
                    else:
                        nc.vector.tensor_tensor(v_sb[tt][:], v_sb[tt][:],
                                                vacc[:], op=ADD)
                if ps == 1:
                    rope0(part, h)
    return wv_sb


def _attention_head(nc, pools, qk_sb, v_sb, consts, jb, h, interleave,
                    head_start=None):
    """Scores/exp/mask/AV/l for one (jb, h), with score pipelining and
    outproj interleave. head_start (the previous head's deferred
    normalize chain) is emitted after this head's first two scores so
    its PE/DVE ops hide behind fresh score work."""
    pss, psc, psl, ppool, rrpool = pools
    tri_sb, onescol_sb = consts
    qT = qk_sb[("q", h)]
    qsl = slice(jb * 512, (jb + 1) * 512)
    nt = 4 * (jb + 1)
    ctx_ps = psc.tile([P, 512], F32, tag="ctx", bufs=2)
    l_ps = psl.tile([1, 512], F32, tag="l", bufs=1)

    def score(i):
        r = i - 4 * jb
        c0 = max(0, r * P)
        osl = slice(c0, 512)
        sps = pss.tile([P, 512], F32, tag="s", bufs=2)
        kT = qk_sb[("k", h)]
        nc.tensor.matmul(sps[:, osl], kT[:, i * P:(i + 1) * P],
                         qT[:, jb * 512 + c0:(jb + 1) * 512],
                         start=True, stop=True)
        pt = ppool.tile([P, 512], BF, tag="pt", bufs=5)
        nc.scalar.activation(pt[:, osl], sps[:, osl], EXP, scale=ESC)
        if r >= 0:
            dsl = slice(r * P, (r + 1) * P)
            nc.gpsimd.tensor_tensor(pt[:, dsl], pt[:, dsl], tri_sb[:], op=MULT)
        return pt, c0

    ahead = 1
    queue = [score(i) for i in range(min(ahead, nt))]
    for i in range(nt):
        pt, c0 = queue.pop(0)
        if i + ahead < nt:
            queue.append(score(i + ahead))
        if i == 0 and head_start is not None:
            head_start()
        osl = slice(c0, 512)
        nc.tensor.matmul(ctx_ps[:, osl], v_sb[i][:, h * P:(h + 1) * P],
                         pt[:, osl], start=(i == 0), stop=(i == nt - 1))
        nc.tensor.matmul(l_ps[:, osl], onescol_sb[:], pt[:, osl],
                         start=(i == 0), stop=(i == nt - 1))
        interleave()
    rinv_row = rrpool.tile([1, 512], BF, tag="rr", bufs=2)
    with nc.allow_low_precision(reason="softmax 1/l bf16"):
        nc.vector.reciprocal(rinv_row[:], l_ps[:])
    return ctx_ps, rinv_row


def _normalize_ctx(nc, pools, consts, ctx_ps, rinv_row, c1t, c2t, sl):
    """broadcast 1/l -> normalize -> split ctx into fp8 hi/lo pair slots."""
    psb, rrpool, cxspool, cxnpool = pools
    onesrow_sb, = consts
    bps = rrpool.tile([P, 512], BF, tag="bb", bufs=2, name="bps_sb")
    nc.gpsimd.partition_broadcast(bps[:], rinv_row[:])
    cvt = cxspool.tile([P, 512], F32, tag="cvt", bufs=2)
    nc.vector.tensor_copy(cvt[:], ctx_ps[:])
    ctxn = cxnpool.tile([P, 512], F32, tag="cxn", bufs=2)
    nc.vector.tensor_tensor(ctxn[:], cvt[:], bps[:], op=MULT)
    nc.vector.tensor_copy(c1t[:, sl], ctxn[:])
    nc.vector.tensor_tensor(c2t[:, sl], ctxn[:], c1t[:, sl], op=SUB)


def _phase2(nc, tc, dram, qk_sb, v_sb, gtiles, vdefer):
    wo1, wo2, y = dram
    tri_sb, onescol_sb, onesrow_sb = gtiles
    with (
        tc.tile_pool(name="wop", bufs=1) as wopool,
        tc.tile_pool(name="ctx1p", bufs=2) as c1pool,
        tc.tile_pool(name="ctx2p", bufs=2) as c2pool,
        tc.tile_pool(name="cxs", bufs=2) as cxspool,
        tc.tile_pool(name="cxn", bufs=2) as cxnpool,
        tc.tile_pool(name="rrow", bufs=2) as rrpool,
        tc.tile_pool(name="pp", bufs=4) as ppool,
        tc.tile_pool(name="yp", bufs=3) as ypool,
        tc.tile_pool(name="pss", bufs=2, space="PSUM") as pss,
        tc.tile_pool(name="psm", bufs=1, space="PSUM") as psm,
        tc.tile_pool(name="psc", bufs=2, space="PSUM") as psc,
        tc.tile_pool(name="psy", bufs=3, space="PSUM") as psy,
    ):
        wo_sb = {}

        def load_wo():
            for hp in range(2):
                for lv, ten in ((1, wo1), (2, wo2)):
                    t_ = wopool.tile([P, 2, C], F8, tag=f"wo{hp}_{lv}")
                    nc.sync.dma_start(t_[:], ten[hp])
                    wo_sb[(hp, lv)] = t_

        ctx1 = {}   # (jb, hp) -> [P, 2, 512] fp8
        ctx2 = {}
        yrow = {}

        def outproj_thunk(jb, sub, ob):
            def run():
                tt = jb * 4 + sub
                ssl = slice(sub * P, (sub + 1) * P)
                osl = slice(ob * 512, (ob + 1) * 512)
                yps = psy.tile([P, 512], F32, tag="y", bufs=3)
                n = 0
                for hp in range(2):
                    for ct, wl in ((ctx1, 1), (ctx1, 2), (ctx2, 1)):
                        nc.tensor.matmul(
                            yps[:], ct[(jb, hp)][:, :, ssl],
                            wo_sb[(hp, wl)][:, :, osl],
                            start=(n == 0), stop=(n == 5), perf_mode=DR)
                        n += 1
                if ob == 0:
                    yrow[tt] = ypool.tile([P, T], BF, tag="ysb", bufs=2,
                                          name=f"yrow{tt}")
                y_sb = yrow[tt]
                if (sub + ob) % 2 == 0:
                    nc.vector.tensor_scalar_mul(y_sb[:, osl], yps[:], 1.0 / WS)
                else:
                    nc.scalar.mul(y_sb[:, osl], yps[:], 1.0 / WS)
                if jb == NB - 1 and sub == NB - 1:
                    nc.sync.dma_start(y[tt * P:(tt + 1) * P, osl],
                                      y_sb[:, osl])
                elif ob == NB - 1:
                    nc.sync.dma_start(y[tt * P:(tt + 1) * P, :], y_sb[:])
            return run

        xtail, wv_sb = vdefer

        def v_thunk(tt):
            def run():
                ssl = slice(tt * P - 3 * 512, tt * P - 3 * 512 + P)
                vacc = psy.tile([P, 512], F32, tag="y", bufs=3, name="vacc")
                n = 0
                for xl, wl in ((1, 1), (1, 2), (2, 1)):
                    for j in range(2, NPAIR):
                        nc.tensor.matmul(
                            vacc[:], xtail[(xl, j)][:, :, ssl], wv_sb[wl][:, j],
                            start=(n == 0), stop=(n == 17), perf_mode=DR)
                        n += 1
                nc.vector.tensor_tensor(v_sb[tt][:], v_sb[tt][:], vacc[:],
                                        op=ADD)
            return run

        pending = [v_thunk(tt) for tt in range(12, NTT)]
        ahead = {"pend": pending, "it": 0, "kint": 0}

        def interleave():
            ahead["it"] += 1
            if ahead["pend"] and ahead["kint"] and \
                    ahead["it"] % ahead["kint"] == 0:
                ahead["pend"].pop(0)()

        att_pools = (pss, psc, psm, ppool, rrpool)
        att_consts = (tri_sb, onescol_sb)
        nrm_pools = (None, rrpool, cxspool, cxnpool)
        nrm_consts = (onesrow_sb,)

        load_wo()
        xfin = [None]

        for jb in range(NB):
            if xfin[0] is not None:
                xfin[0]()
                xfin[0] = None

            nt = 4 * (jb + 1)
            ahead["it"] = 0
            ahead["kint"] = (HPC * nt) // len(pending) if pending else 0

            fin = None
            for h in range(HPC):
                ctx_ps, rinv_row = _attention_head(
                    nc, att_pools, qk_sb, v_sb, att_consts, jb, h, interleave,
                    head_start=fin)
                hp, sl = h // 2, h % 2
                if sl == 0:
                    ctx1[(jb, hp)] = c1pool.tile(
                        [P, 2, 512], F8, tag=f"c1_{hp}", bufs=2,
                        name=f"c1_{jb}_{hp}")
                    ctx2[(jb, hp)] = c2pool.tile(
                        [P, 2, 512], F8, tag=f"c2_{hp}", bufs=2,
                        name=f"c2_{jb}_{hp}")

                def fin(cp=ctx_ps, rr=rinv_row, c1t=ctx1[(jb, hp)],
                        c2t=ctx2[(jb, hp)], s=sl):
                    _normalize_ctx(nc, nrm_pools, nrm_consts, cp, rr,
                                   c1t, c2t, s)
            if jb + 1 < NB:
                xfin[0] = fin
            else:
                fin()

            while pending:
                pending.pop(0)()
            pending.extend(outproj_thunk(jb, sub, ob)
                           for sub in range(4) for ob in range(4))
            ahead["pend"] = pending

        while pending:
            pending.pop(0)()


def _build():
    nc = bacc.Bacc("TRN2", target_bir_lowering=False, debug=False,
                   num_devices=NCORES)
    x1 = nc.dram_tensor("x1", (NPAIR, P, 2, T), F8, kind="ExternalInput").ap()
    x2 = nc.dram_tensor("x2", (NPAIR, P, 2, T), F8, kind="ExternalInput").ap()
    wq1 = nc.dram_tensor("wq1", (HPC, P, NPAIR, 2, P), F8, kind="ExternalInput").ap()
    wq2 = nc.dram_tensor("wq2", (HPC, P, NPAIR, 2, P), F8, kind="ExternalInput").ap()
    wk1 = nc.dram_tensor("wk1", (HPC, P, NPAIR, 2, P), F8, kind="ExternalInput").ap()
    wk2 = nc.dram_tensor("wk2", (HPC, P, NPAIR, 2, P), F8, kind="ExternalInput").ap()
    wv1 = nc.dram_tensor("wv1", (P, NPAIR, 2, HPC * P), F8, kind="ExternalInput").ap()
    wv2 = nc.dram_tensor("wv2", (P, NPAIR, 2, HPC * P), F8, kind="ExternalInput").ap()
    wo1 = nc.dram_tensor("wo1", (2, P, 2, C), F8, kind="ExternalInput").ap()
    wo2 = nc.dram_tensor("wo2", (2, P, 2, C), F8, kind="ExternalInput").ap()
    cosT = nc.dram_tensor("cosT", (P, T), BF, kind="ExternalInput").ap()
    sinT = nc.dram_tensor("sinT", (P, T), BF, kind="ExternalInput").ap()
    tri = nc.dram_tensor("tri", (P, P), BF, kind="ExternalInput").ap()
    onescol = nc.dram_tensor("onescol", (P, 1), BF, kind="ExternalInput").ap()
    onesrow = nc.dram_tensor("onesrow", (1, P), BF, kind="ExternalInput").ap()
    y = nc.dram_tensor("y", (T, C), BF, kind="ExternalOutput").ap()

    with tile.TileContext(nc) as tc:
        with (
            tc.tile_pool(name="gconst", bufs=1) as gpool,
            tc.tile_pool(name="qkbuf", bufs=1) as qkpool,
            tc.tile_pool(name="vbuf", bufs=1) as vpool,
        ):
            tri_sb = gpool.tile([P, P], BF, tag="tri")
            onescol_sb = gpool.tile([P, 1], BF, tag="ocol")
            onesrow_sb = gpool.tile([1, P], BF, tag="orow")
            cos_sb = gpool.tile([P, T], BF, tag="cos")
            sin_sb = gpool.tile([P, T], BF, tag="sin")

            def const_loads():
                nc.sync.dma_start(cos_sb[:], cosT)
                nc.sync.dma_start(sin_sb[:], sinT)
                nc.sync.dma_start(tri_sb[:], tri)
                nc.sync.dma_start(onescol_sb[:], onescol)
                nc.sync.dma_start(onesrow_sb[:], onesrow)

            wvpool_cm = tc.tile_pool(name="wvp", bufs=1)
            wvpool = wvpool_cm.__enter__()
            xtpool_cm = tc.tile_pool(name="xtail", bufs=1)
            xtpool = xtpool_cm.__enter__()
            xtail = {}

            def xtail_loads():
                for j in range(2, NPAIR):
                    for lv, ten in ((1, x1), (2, x2)):
                        t_ = xtpool.tile([P, 2, 512], F8, tag=f"xt{lv}_{j}",
                                         name=f"xt{lv}_{j}")
                        nc.sync.dma_start(t_[:], ten[j][:, :, 3 * 512:])
                        xtail[(lv, j)] = t_

            spool_cm = tc.tile_pool(name="st", bufs=2)
            spool = spool_cm.__enter__()
            half = P // 2

            def rope_row(part, h):
                # whole-row rope for (part, h): the half-swap is 2 big DMAs
                # instead of 8 small ones (descriptor floor dominates small
                # transfers), and the mul/mul/add chain is 3 ops per row
                src = qk_sb[(part, h)]
                tmp = spool.tile([P, T], BF, tag="rt", bufs=2, name="rtmp")
                t1 = spool.tile([P, T], BF, tag="t1", bufs=2, name="rt1")
                t2 = spool.tile([P, T], BF, tag="t2", bufs=2, name="rt2")
                nc.sync.dma_start(tmp[0:half, :], src[half:P, :])
                nc.sync.dma_start(tmp[half:P, :], src[0:half, :])
                nc.gpsimd.tensor_tensor(t1[:], src[:], cos_sb[:], op=MULT)
                nc.vector.tensor_tensor(t2[:], tmp[:], sin_sb[:], op=MULT)
                nc.vector.tensor_tensor(src[:], t1[:], t2[:], op=ADD)

            qk_sb = {}
            for h in range(HPC):
                for part in ("q", "k"):
                    qk_sb[(part, h)] = qkpool.tile(
                        [P, T], BF, tag=f"{part}{h}",
                        name=f"{part}{h}_sb")
            v_sb = [vpool.tile([P, HPC * P], BF, tag=f"vb{i}", name=f"v{i}_sb")
                    for i in range(NTT)]

            wv_sb = _phase1(nc, tc, (x1, x2, wq1, wq2, wk1, wk2, wv1, wv2),
                            qk_sb, v_sb, const_loads, rope_row, wvpool,
                            xtail_loads)
            spool_cm.__exit__(None, None, None)
            _phase2(nc, tc, (wo1, wo2, y), qk_sb, v_sb,
                    (tri_sb, onescol_sb, onesrow_sb), (xtail, wv_sb))
            xtpool_cm.__exit__(None, None, None)
            wvpool_cm.__exit__(None, None, None)

    nc.compile()
    return nc


def _build_kernel():
    if "k" not in _CACHE:
        _CACHE["k"] = _build()
    return _CACHE["k"]


E4NP = ml_dtypes.float8_e4m3
BFNP = ml_dtypes.bfloat16


def _split8(a):
    a1 = np.asarray(a, np.float32).astype(E4NP)
    a2 = (np.asarray(a, np.float32) - a1.astype(np.float32)).astype(E4NP)
    return a1, a2


def prepare_in_maps(x, W_qkv, W_o, cos, sin):
    tri01 = (np.arange(P)[:, None] <= np.arange(P)[None, :]).astype(BFNP)
    onescol = np.full((P, 1), WS, dtype=np.float32).astype(BFNP)
    onesrow = np.ones((1, P), dtype=np.float32).astype(BFNP)
    cosT = np.ascontiguousarray(cos.T).astype(BFNP)
    sgn = np.where(np.arange(P) < P // 2, -1.0, 1.0).astype(np.float32)
    sinT = np.ascontiguousarray(sin.T * sgn[:, None]).astype(BFNP)

    # per-batch x fp8 pair chunks [NPAIR, P, 2, T]
    xq = {}
    for b in range(2):
        xT = np.ascontiguousarray(x[b].T)
        xs = _split8(xT)
        xq[b] = tuple(
            np.ascontiguousarray(
                a.reshape(NPAIR, 2, P, T).transpose(0, 2, 1, 3))
            for a in xs)

    def wqk_layout(a):   # [C, 512] -> (HPC, P, NPAIR, 2, P)
        return np.ascontiguousarray(
            a.reshape(NPAIR, 2, P, HPC, P).transpose(3, 2, 0, 1, 4))

    def wv_layout(a):    # [C, 512] -> (P, NPAIR, 2, HPC*P)
        return np.ascontiguousarray(
            a.reshape(NPAIR, 2, P, HPC * P).transpose(2, 0, 1, 3))

    in_maps = []
    for core in range(NCORES):
        b = core // 4
        hg0 = (core % 4) * HPC
        rows = slice(hg0 * P, (hg0 + HPC) * P)
        wq_r = WS * W_qkv[0 * C:1 * C][rows].T   # [C, 512]
        wk_r = WS * W_qkv[1 * C:2 * C][rows].T
        wv_r = WS * W_qkv[2 * C:3 * C][rows].T
        wq1, wq2 = (wqk_layout(a) for a in _split8(wq_r))
        wk1, wk2 = (wqk_layout(a) for a in _split8(wk_r))
        wv1, wv2 = (wv_layout(a) for a in _split8(wv_r))
        wo_r = WS * W_o[:, rows].T               # [512, C]
        wo1, wo2 = (
            np.ascontiguousarray(a.reshape(2, 2, P, C).transpose(0, 2, 1, 3))
            for a in _split8(wo_r))
        in_maps.append({
            "x1": xq[b][0], "x2": xq[b][1],
            "wq1": wq1, "wq2": wq2, "wk1": wk1, "wk2": wk2,
            "wv1": wv1, "wv2": wv2, "wo1": wo1, "wo2": wo2,
            "cosT": cosT, "sinT": sinT, "tri": tri01,
            "onescol": onescol, "onesrow": onesrow,
        })
    return in_maps


def gather(results, b_o):
    y = np.zeros((2, T, C), dtype=np.float32)
    for core in range(NCORES):
        y[core // 4] += np.asarray(results[core]["y"], dtype=np.float32)
    y += np.asarray(b_o, dtype=np.float32)[None, None, :]
    return y


def kernel(x, W_qkv, W_o, b_o, cos, sin):
    x = np.asarray(x, dtype=np.float32)
    W_qkv = np.asarray(W_qkv, dtype=np.float32)
    W_o = np.asarray(W_o, dtype=np.float32)
    cos = np.asarray(cos, dtype=np.float32)
    sin = np.asarray(sin, dtype=np.float32)
    nc = _build_kernel()
    in_maps = prepare_in_maps(x, W_qkv, W_o, cos, sin)
    res = run_bass_kernel_spmd(nc, in_maps, core_ids=list(range(NCORES)))
    return gather(res.results, b_o)


# revision 41
# speedup vs baseline: 1.4370x; 1.0426x over previous
"""Causal multi-head attention (RoPE) on 8 TRN2 NeuronCores.

Problem: x[2,2048,2048] -> qkv proj -> rope -> causal attention (16 heads,
head_dim 128) -> output proj + bias. Sharding: (batch, head-group) across the
8 cores - core c handles batch c//4 and heads 4*(c%4)..4*(c%4)+3. Each core
computes a partial output projection over its heads' channels; the host sums
the 4 partials per batch and adds b_o.

Mixed-precision pipeline (tolerance 2e-2; this lands ~3.5e-3):
  - QKV projection and output projection run in fp8 (e4m3) with a hi/lo
    3-term split (W1X1 + W1X2 + W2X1, weights pre-scaled by 64 into the
    e4m3 normal range) using DoubleRow matmuls: each instruction contracts
    2x128 rows at 0.5 cycles per output column - 2.67x the f32r rate for
    the same accuracy class.
  - Attention (scores, exp, AV) runs in bf16 (1 cyc/col, no 256-col floor,
    so causal narrowing works at 128-col granularity).
  - Softmax row-sums l use pt as the matmul *stationary* operand with a
    [128,1] ones column as the moving operand: cost 1 cycle per tile-chunk
    instead of N. 1/l is transposed back to row form via 4 tiny PE
    transposes + 4 K=1 broadcast matmuls.
  - All evictions/elementwise work spread across Pool/DVE/ACT to keep the
    sidecar engines under the PE roofline.

Layout: all matmuls keep contraction on partitions; q,k produced transposed
[d, tok], v natural [tok, (h,d)]; scores transposed s^T[tk, tq] so softmax
needs no transposes; ctx^T[d, tq] accumulates over tk tiles; outproj
contracts the 4 heads' channels as 2 DoubleRow head-pairs. The output
projection for block jb-1 is interleaved into block jb's attention inner
loops to fill the PE bubbles left by the exp dependency chain.
"""
import math

import numpy as np
import ml_dtypes

import concourse.bacc as bacc
import concourse.mybir as mybir
import concourse.tile as tile
from concourse.bass_utils import run_bass_kernel_spmd

P = 128           # partitions / head_dim
T = 2048          # context length
C = 2048          # d_model
NTT = T // P      # 16 token tiles
NB = T // 512     # 4 query blocks of 512
HPC = 4           # heads per core
NPAIR = C // 256  # 8 DoubleRow contraction pair-chunks
NCORES = 8
WS = 64.0         # fp8 weight pre-scale
SCALE = 1.0 / math.sqrt(P)
ESC = SCALE / (WS * WS)   # exp() scale: scores carry WS^2

F32 = mybir.dt.float32
BF = mybir.dt.bfloat16
F8 = mybir.dt.float8e4
EXP = mybir.ActivationFunctionType.Exp
MULT = mybir.AluOpType.mult
ADD = mybir.AluOpType.add
SUB = mybir.AluOpType.subtract
DR = mybir.MatmulPerfMode.DoubleRow

_CACHE = {}


def _phase1(nc, tc, dram, qk_sb, v_sb, const_loads, rope0, wvpool,
            xtail_loads):
    """QKV projection: fp8 hi/lo 3-term DoubleRow. Pass A covers pair-chunks
    0-1 (so PE starts ~4us in, DMA-paced), pass B covers 2-7; pass-B partials
    merge into bf16 SBUF via DVE adds. rope0(part, h) is invoked after each
    pass-B group so block-0 rope chains run during pass B."""
    x1, x2, wq1, wq2, wk1, wk2, wv1, wv2 = dram
    PASSES = (list(range(0, 2)), list(range(2, 8)))
    with (
        tc.tile_pool(name="xp", bufs=1) as xpool,
        tc.tile_pool(name="wp", bufs=1) as wpool,
        tc.tile_pool(name="psqk", bufs=4, space="PSUM") as psqk,
        tc.tile_pool(name="psv", bufs=3, space="PSUM") as psv,
    ):
        wten = {"q": (wq1, wq2), "k": (wk1, wk2)}
        groups = [(h, part) for h in range(HPC) for part in ("q", "k")]

        def load_w(gi, ps):
            h, part = groups[gi]
            js = PASSES[ps]
            jsl = slice(js[0], js[-1] + 1)
            w_sb = {}
            for lv in (1, 2):
                t_ = wpool.tile([P, len(js), 2, P], F8, tag=f"w{ps}_{gi}_{lv}",
                                bufs=1, name=f"w{part}{h}p{ps}_{lv}")
                nc.sync.dma_start(t_[:], wten[part][lv - 1][h][:, jsl])
                w_sb[lv] = t_
            return w_sb

        wq_pref = {(0, 0): load_w(0, 0)}
        xt = {}

        def load_x(js_):
            for j in js_:
                for lv, ten in ((1, x1), (2, x2)):
                    t_ = xpool.tile([P, 2, T], F8, tag=f"x{lv}_{j}",
                                    bufs=1, name=f"x{lv}_{j}")
                    nc.sync.dma_start(t_[:], ten[j])
                    xt[(lv, j)] = t_

        load_x(PASSES[0])
        for gi in range(1, len(groups)):
            wq_pref[(gi, 0)] = load_w(gi, 0)
        wv_sb = {}
        for lv, ten in ((1, wv1), (2, wv2)):
            t_ = wvpool.tile([P, NPAIR, 2, HPC * P], F8, tag=f"wv{lv}")
            nc.sync.dma_start(t_[:], ten)
            wv_sb[lv] = t_
        load_x(PASSES[1])
        const_loads()
        xtail_loads()
        for gi in range(len(groups)):
            wq_pref[(gi, 1)] = load_w(gi, 1)

        for ps in range(2):
            js = PASSES[ps]
            nmm = 3 * len(js)
            for gi, (h, part) in enumerate(groups):
                w_sb = wq_pref.pop((gi, ps))
                for nb in range(NB):
                    tsl = slice(nb * 512, (nb + 1) * 512)
                    acc = psqk.tile([P, 512], F32, tag="qk", bufs=4)
                    n = 0
                    order = ([(jj, t) for jj in range(len(js)) for t in range(3)]
                             if ps == 0 else
                             [(jj, t) for t in range(3) for jj in range(len(js))])
                    terms = ((1, 1), (1, 2), (2, 1))
                    for jj, t in order:
                        wl, xl = terms[t]
                        nc.tensor.matmul(
                            acc[:], w_sb[wl][:, jj], xt[(xl, js[jj])][:, :, tsl],
                            start=(n == 0), stop=(n == nmm - 1),
                            perf_mode=DR)
                        n += 1
                    dst = qk_sb[(part, h)]
                    if ps == 0:
                        nc.scalar.copy(dst[:, tsl], acc[:])
                    else:
                        nc.vector.tensor_tensor(dst[:, tsl], dst[:, tsl],
                                                acc[:], op=ADD)
                # two v token-tiles after each q/k group (the last four
                # pass-B tiles are deferred into block-0 attention)
                for tt in (2 * gi, 2 * gi + 1):
                    if ps == 1 and tt >= 12:
                        continue
                    ssl = slice(tt * P, (tt + 1) * P)
                    vacc = psv.tile([P, 512], F32, tag="v", bufs=3)
                    n = 0
                    for xl, wl in ((1, 1), (1, 2), (2, 1)):
                        for jj, j in enumerate(js):
                            nc.tensor.matmul(
                                vacc[:], xt[(xl, j)][:, :, ssl], wv_sb[wl][:, j],
                                start=(n == 0), stop=(n == nmm - 1),
                                perf_mode=DR)
                            n += 1
                    if ps == 0:
                        nc.scalar.copy(v_sb[tt][:], vacc[:])
                    else:
                        nc.vector.tensor_tensor(v_sb[tt][:], v_sb[tt][:],
                                                vacc[:], op=ADD)
                if ps == 1:
                    rope0(part, h)
    return wv_sb


def _attention_head(nc, pools, qk_sb, v_sb, consts, jb, h, interleave,
                    head_start=None):
    """Scores/exp/mask/AV/l for one (jb, h), with score pipelining and
    outproj interleave. head_start (the previous head's deferred
    normalize chain) is emitted after this head's first two scores so
    its PE/DVE ops hide behind fresh score work."""
    pss, psc, psl, ppool, rrpool = pools
    tri_sb, onescol_sb = consts
    qT = qk_sb[("q", h)]
    qsl = slice(jb * 512, (jb + 1) * 512)
    nt = 4 * (jb + 1)
    ctx_ps = psc.tile([P, 512], F32, tag="ctx", bufs=2)
    l_ps = psl.tile([1, 512], F32, tag="l", bufs=1)

    def score(i):
        r = i - 4 * jb
        c0 = max(0, r * P)
        osl = slice(c0, 512)
        sps = pss.tile([P, 512], F32, tag="s", bufs=2)
        kT = qk_sb[("k", h)]
        nc.tensor.matmul(sps[:, osl], kT[:, i * P:(i + 1) * P],
                         qT[:, jb * 512 + c0:(jb + 1) * 512],
                         start=True, stop=True)
        pt = ppool.tile([P, 512], BF, tag="pt", bufs=5)
        nc.scalar.activation(pt[:, osl], sps[:, osl], EXP, scale=ESC)
        if r >= 0:
            dsl = slice(r * P, (r + 1) * P)
            nc.gpsimd.tensor_tensor(pt[:, dsl], pt[:, dsl], tri_sb[:], op=MULT)
        return pt, c0

    ahead = 4
    queue = [score(i) for i in range(min(ahead, nt))]
    for i in range(nt):
        pt, c0 = queue.pop(0)
        if i + ahead < nt:
            queue.append(score(i + ahead))
        if i == 0 and head_start is not None:
            head_start()
        osl = slice(c0, 512)
        nc.tensor.matmul(ctx_ps[:, osl], v_sb[i][:, h * P:(h + 1) * P],
                         pt[:, osl], start=(i == 0), stop=(i == nt - 1))
        nc.tensor.matmul(l_ps[:, osl], onescol_sb[:], pt[:, osl],
                         start=(i == 0), stop=(i == nt - 1))
        interleave()
    rinv_row = rrpool.tile([1, 512], BF, tag="rr", bufs=2)
    with nc.allow_low_precision(reason="softmax 1/l bf16"):
        nc.vector.reciprocal(rinv_row[:], l_ps[:])
    return ctx_ps, rinv_row


def _normalize_ctx(nc, pools, consts, ctx_ps, rinv_row, c1t, c2t, sl):
    """broadcast 1/l -> normalize -> split ctx into fp8 hi/lo pair slots."""
    psb, rrpool, cxspool, cxnpool = pools
    onesrow_sb, = consts
    bps = rrpool.tile([P, 512], BF, tag="bb", bufs=2, name="bps_sb")
    nc.gpsimd.partition_broadcast(bps[:], rinv_row[:])
    cvt = cxspool.tile([P, 512], F32, tag="cvt", bufs=2)
    nc.vector.tensor_copy(cvt[:], ctx_ps[:])
    ctxn = cxnpool.tile([P, 512], F32, tag="cxn", bufs=2)
    nc.vector.tensor_tensor(ctxn[:], cvt[:], bps[:], op=MULT)
    nc.vector.tensor_copy(c1t[:, sl], ctxn[:])
    nc.vector.tensor_tensor(c2t[:, sl], ctxn[:], c1t[:, sl], op=SUB)


def _phase2(nc, tc, dram, qk_sb, v_sb, gtiles, vdefer):
    wo1, wo2, y = dram
    tri_sb, onescol_sb, onesrow_sb = gtiles
    with (
        tc.tile_pool(name="wop", bufs=1) as wopool,
        tc.tile_pool(name="ctx1p", bufs=2) as c1pool,
        tc.tile_pool(name="ctx2p", bufs=2) as c2pool,
        tc.tile_pool(name="cxs", bufs=2) as cxspool,
        tc.tile_pool(name="cxn", bufs=2) as cxnpool,
        tc.tile_pool(name="rrow", bufs=2) as rrpool,
        tc.tile_pool(name="pp", bufs=4) as ppool,
        tc.tile_pool(name="yp", bufs=3) as ypool,
        tc.tile_pool(name="pss", bufs=2, space="PSUM") as pss,
        tc.tile_pool(name="psm", bufs=1, space="PSUM") as psm,
        tc.tile_pool(name="psc", bufs=2, space="PSUM") as psc,
        tc.tile_pool(name="psy", bufs=3, space="PSUM") as psy,
    ):
        wo_sb = {}

        def load_wo():
            for hp in range(2):
                for lv, ten in ((1, wo1), (2, wo2)):
                    t_ = wopool.tile([P, 2, C], F8, tag=f"wo{hp}_{lv}")
                    nc.sync.dma_start(t_[:], ten[hp])
                    wo_sb[(hp, lv)] = t_

        ctx1 = {}   # (jb, hp) -> [P, 2, 512] fp8
        ctx2 = {}
        yrow = {}

        def outproj_thunk(jb, sub, ob):
            def run():
                tt = jb * 4 + sub
                ssl = slice(sub * P, (sub + 1) * P)
                osl = slice(ob * 512, (ob + 1) * 512)
                yps = psy.tile([P, 512], F32, tag="y", bufs=3)
                n = 0
                for hp in range(2):
                    for ct, wl in ((ctx1, 1), (ctx1, 2), (ctx2, 1)):
                        nc.tensor.matmul(
                            yps[:], ct[(jb, hp)][:, :, ssl],
                            wo_sb[(hp, wl)][:, :, osl],
                            start=(n == 0), stop=(n == 5), perf_mode=DR)
                        n += 1
                if ob == 0:
                    yrow[tt] = ypool.tile([P, T], BF, tag="ysb", bufs=2,
                                          name=f"yrow{tt}")
                y_sb = yrow[tt]
                if (sub + ob) % 2 == 0:
                    nc.vector.tensor_scalar_mul(y_sb[:, osl], yps[:], 1.0 / WS)
                else:
                    nc.scalar.mul(y_sb[:, osl], yps[:], 1.0 / WS)
                if jb == NB - 1 and sub == NB - 1:
                    nc.sync.dma_start(y[tt * P:(tt + 1) * P, osl],
                                      y_sb[:, osl])
                elif ob == NB - 1:
                    nc.sync.dma_start(y[tt * P:(tt + 1) * P, :], y_sb[:])
            return run

        xtail, wv_sb = vdefer

        def v_thunk(tt):
            def run():
                ssl = slice(tt * P - 3 * 512, tt * P - 3 * 512 + P)
                vacc = psy.tile([P, 512], F32, tag="y", bufs=3, name="vacc")
                n = 0
                for xl, wl in ((1, 1), (1, 2), (2, 1)):
                    for j in range(2, NPAIR):
                        nc.tensor.matmul(
                            vacc[:], xtail[(xl, j)][:, :, ssl], wv_sb[wl][:, j],
                            start=(n == 0), stop=(n == 17), perf_mode=DR)
                        n += 1
                nc.vector.tensor_tensor(v_sb[tt][:], v_sb[tt][:], vacc[:],
                                        op=ADD)
            return run

        pending = [v_thunk(tt) for tt in range(12, NTT)]
        ahead = {"pend": pending, "it": 0, "kint": 0}

        def interleave():
            ahead["it"] += 1
            if ahead["pend"] and ahead["kint"] and \
                    ahead["it"] % ahead["kint"] == 0:
                ahead["pend"].pop(0)()

        att_pools = (pss, psc, psm, ppool, rrpool)
        att_consts = (tri_sb, onescol_sb)
        nrm_pools = (None, rrpool, cxspool, cxnpool)
        nrm_consts = (onesrow_sb,)

        load_wo()
        xfin = [None]

        for jb in range(NB):
            if xfin[0] is not None:
                xfin[0]()
                xfin[0] = None

            nt = 4 * (jb + 1)
            ahead["it"] = 0
            ahead["kint"] = (HPC * nt) // len(pending) if pending else 0

            fin = None
            for h in range(HPC):
                ctx_ps, rinv_row = _attention_head(
                    nc, att_pools, qk_sb, v_sb, att_consts, jb, h, interleave,
                    head_start=fin)
                hp, sl = h // 2, h % 2
                if sl == 0:
                    ctx1[(jb, hp)] = c1pool.tile(
                        [P, 2, 512], F8, tag=f"c1_{hp}", bufs=2,
                        name=f"c1_{jb}_{hp}")
                    ctx2[(jb, hp)] = c2pool.tile(
                        [P, 2, 512], F8, tag=f"c2_{hp}", bufs=2,
                        name=f"c2_{jb}_{hp}")

                def fin(cp=ctx_ps, rr=rinv_row, c1t=ctx1[(jb, hp)],
                        c2t=ctx2[(jb, hp)], s=sl):
                    _normalize_ctx(nc, nrm_pools, nrm_consts, cp, rr,
                                   c1t, c2t, s)
            if jb + 1 < NB:
                xfin[0] = fin
            else:
                fin()

            while pending:
                pending.pop(0)()
            pending.extend(outproj_thunk(jb, sub, ob)
                           for sub in range(4) for ob in range(4))
            ahead["pend"] = pending

        while pending:
            pending.pop(0)()


def _build():
    nc = bacc.Bacc("TRN2", target_bir_lowering=False, debug=False,
                   num_devices=NCORES)
    x1 = nc.dram_tensor("x1", (NPAIR, P, 2, T), F8, kind="ExternalInput").ap()
    x2 = nc.dram_tensor("x2", (NPAIR, P, 2, T), F8, kind="ExternalInput").ap()
    wq1 = nc.dram_tensor("wq1", (HPC, P, NPAIR, 2, P), F8, kind="ExternalInput").ap()
    wq2 = nc.dram_tensor("wq2", (HPC, P, NPAIR, 2, P), F8, kind="ExternalInput").ap()
    wk1 = nc.dram_tensor("wk1", (HPC, P, NPAIR, 2, P), F8, kind="ExternalInput").ap()
    wk2 = nc.dram_tensor("wk2", (HPC, P, NPAIR, 2, P), F8, kind="ExternalInput").ap()
    wv1 = nc.dram_tensor("wv1", (P, NPAIR, 2, HPC * P), F8, kind="ExternalInput").ap()
    wv2 = nc.dram_tensor("wv2", (P, NPAIR, 2, HPC * P), F8, kind="ExternalInput").ap()
    wo1 = nc.dram_tensor("wo1", (2, P, 2, C), F8, kind="ExternalInput").ap()
    wo2 = nc.dram_tensor("wo2", (2, P, 2, C), F8, kind="ExternalInput").ap()
    cosT = nc.dram_tensor("cosT", (P, T), BF, kind="ExternalInput").ap()
    sinT = nc.dram_tensor("sinT", (P, T), BF, kind="ExternalInput").ap()
    tri = nc.dram_tensor("tri", (P, P), BF, kind="ExternalInput").ap()
    onescol = nc.dram_tensor("onescol", (P, 1), BF, kind="ExternalInput").ap()
    onesrow = nc.dram_tensor("onesrow", (1, P), BF, kind="ExternalInput").ap()
    y = nc.dram_tensor("y", (T, C), BF, kind="ExternalOutput").ap()

    with tile.TileContext(nc) as tc:
        with (
            tc.tile_pool(name="gconst", bufs=1) as gpool,
            tc.tile_pool(name="qkbuf", bufs=1) as qkpool,
            tc.tile_pool(name="vbuf", bufs=1) as vpool,
        ):
            tri_sb = gpool.tile([P, P], BF, tag="tri")
            onescol_sb = gpool.tile([P, 1], BF, tag="ocol")
            onesrow_sb = gpool.tile([1, P], BF, tag="orow")
            cos_sb = gpool.tile([P, T], BF, tag="cos")
            sin_sb = gpool.tile([P, T], BF, tag="sin")

            def const_loads():
                nc.sync.dma_start(cos_sb[:], cosT)
                nc.sync.dma_start(sin_sb[:], sinT)
                nc.sync.dma_start(tri_sb[:], tri)
                nc.sync.dma_start(onescol_sb[:], onescol)
                nc.sync.dma_start(onesrow_sb[:], onesrow)

            wvpool_cm = tc.tile_pool(name="wvp", bufs=1)
            wvpool = wvpool_cm.__enter__()
            xtpool_cm = tc.tile_pool(name="xtail", bufs=1)
            xtpool = xtpool_cm.__enter__()
            xtail = {}

            def xtail_loads():
                for j in range(2, NPAIR):
                    for lv, ten in ((1, x1), (2, x2)):
                        t_ = xtpool.tile([P, 2, 512], F8, tag=f"xt{lv}_{j}",
                                         name=f"xt{lv}_{j}")
                        nc.sync.dma_start(t_[:], ten[j][:, :, 3 * 512:])
                        xtail[(lv, j)] = t_

            spool_cm = tc.tile_pool(name="st", bufs=2)
            spool = spool_cm.__enter__()
            half = P // 2

            def rope_row(part, h):
                # whole-row rope for (part, h): the half-swap is 2 big DMAs
                # instead of 8 small ones (descriptor floor dominates small
                # transfers), and the mul/mul/add chain is 3 ops per row
                src = qk_sb[(part, h)]
                tmp = spool.tile([P, T], BF, tag="rt", bufs=2, name="rtmp")
                t1 = spool.tile([P, T], BF, tag="t1", bufs=2, name="rt1")
                t2 = spool.tile([P, T], BF, tag="t2", bufs=2, name="rt2")
                nc.sync.dma_start(tmp[0:half, :], src[half:P, :])
                nc.sync.dma_start(tmp[half:P, :], src[0:half, :])
                nc.gpsimd.tensor_tensor(t1[:], src[:], cos_sb[:], op=MULT)
                nc.vector.tensor_tensor(t2[:], tmp[:], sin_sb[:], op=MULT)
                nc.vector.tensor_tensor(src[:], t1[:], t2[:], op=ADD)

            qk_sb = {}
            for h in range(HPC):
                for part in ("q", "k"):
                    qk_sb[(part, h)] = qkpool.tile(
                        [P, T], BF, tag=f"{part}{h}",
                        name=f"{part}{h}_sb")
            v_sb = [vpool.tile([P, HPC * P], BF, tag=f"vb{i}", name=f"v{i}_sb")
                    for i in range(NTT)]

            wv_sb = _phase1(nc, tc, (x1, x2, wq1, wq2, wk1, wk2, wv1, wv2),
                            qk_sb, v_sb, const_loads, rope_row, wvpool,
                            xtail_loads)
            spool_cm.__exit__(None, None, None)
            _phase2(nc, tc, (wo1, wo2, y), qk_sb, v_sb,
                    (tri_sb, onescol_sb, onesrow_sb), (xtail, wv_sb))
            xtpool_cm.__exit__(None, None, None)
            wvpool_cm.__exit__(None, None, None)

    nc.compile()
    return nc


def _build_kernel():
    if "k" not in _CACHE:
        _CACHE["k"] = _build()
    return _CACHE["k"]


E4NP = ml_dtypes.float8_e4m3
BFNP = ml_dtypes.bfloat16


def _split8(a):
    a1 = np.asarray(a, np.float32).astype(E4NP)
    a2 = (np.asarray(a, np.float32) - a1.astype(np.float32)).astype(E4NP)
    return a1, a2


def prepare_in_maps(x, W_qkv, W_o, cos, sin):
    tri01 = (np.arange(P)[:, None] <= np.arange(P)[None, :]).astype(BFNP)
    onescol = np.full((P, 1), WS, dtype=np.float32).astype(BFNP)
    onesrow = np.ones((1, P), dtype=np.float32).astype(BFNP)
    cosT = np.ascontiguousarray(cos.T).astype(BFNP)
    sgn = np.where(np.arange(P) < P // 2, -1.0, 1.0).astype(np.float32)
    sinT = np.ascontiguousarray(sin.T * sgn[:, None]).astype(BFNP)

    # per-batch x fp8 pair chunks [NPAIR, P, 2, T]
    xq = {}
    for b in range(2):
        xT = np.ascontiguousarray(x[b].T)
        xs = _split8(xT)
        xq[b] = tuple(
            np.ascontiguousarray(
                a.reshape(NPAIR, 2, P, T).transpose(0, 2, 1, 3))
            for a in xs)

    def wqk_layout(a):   # [C, 512] -> (HPC, P, NPAIR, 2, P)
        return np.ascontiguousarray(
            a.reshape(NPAIR, 2, P, HPC, P).transpose(3, 2, 0, 1, 4))

    def wv_layout(a):    # [C, 512] -> (P, NPAIR, 2, HPC*P)
        return np.ascontiguousarray(
            a.reshape(NPAIR, 2, P, HPC * P).transpose(2, 0, 1, 3))

    in_maps = []
    for core in range(NCORES):
        b = core // 4
        hg0 = (core % 4) * HPC
        rows = slice(hg0 * P, (hg0 + HPC) * P)
        wq_r = WS * W_qkv[0 * C:1 * C][rows].T   # [C, 512]
        wk_r = WS * W_qkv[1 * C:2 * C][rows].T
        wv_r = WS * W_qkv[2 * C:3 * C][rows].T
        wq1, wq2 = (wqk_layout(a) for a in _split8(wq_r))
        wk1, wk2 = (wqk_layout(a) for a in _split8(wk_r))
        wv1, wv2 = (wv_layout(a) for a in _split8(wv_r))
        wo_r = WS * W_o[:, rows].T               # [512, C]
        wo1, wo2 = (
            np.ascontiguousarray(a.reshape(2, 2, P, C).transpose(0, 2, 1, 3))
            for a in _split8(wo_r))
        in_maps.append({
            "x1": xq[b][0], "x2": xq[b][1],
            "wq1": wq1, "wq2": wq2, "wk1": wk1, "wk2": wk2,
            "wv1": wv1, "wv2": wv2, "wo1": wo1, "wo2": wo2,
            "cosT": cosT, "sinT": sinT, "tri": tri01,
            "onescol": onescol, "onesrow": onesrow,
        })
    return in_maps


def gather(results, b_o):
    y = np.zeros((2, T, C), dtype=np.float32)
    for core in range(NCORES):
        y[core // 4] += np.asarray(results[core]["y"], dtype=np.float32)
    y += np.asarray(b_o, dtype=np.float32)[None, None, :]
    return y


def kernel(x, W_qkv, W_o, b_o, cos, sin):
    x = np.asarray(x, dtype=np.float32)
    W_qkv = np.asarray(W_qkv, dtype=np.float32)
    W_o = np.asarray(W_o, dtype=np.float32)
    cos = np.asarray(cos, dtype=np.float32)
    sin = np.asarray(sin, dtype=np.float32)
    nc = _build_kernel()
    in_maps = prepare_in_maps(x, W_qkv, W_o, cos, sin)
    res = run_bass_kernel_spmd(nc, in_maps, core_ids=list(range(NCORES)))
    return gather(res.results, b_o)
